# revision 59
# baseline (speedup 1.0000x reference)
"""DeBERTa-bare Trainium2 Bass kernel.

Topology: 8 NeuronCores = 4 data-parallel pairs (one batch element each) x
2-way tensor parallel (heads + FFN split) with pairwise AllReduce.

Everything on-chip runs feature-major ("transposed"): h is kept as
hT[d, token].  The DeBERTa disentangled-attention gathers
(take_along_axis over relative positions) are realized as affine "skew"
access-pattern DMA reads from DRAM-resident, clamp-extended c2p/p2c tables
(fp8, x256 scaled), injected into the score PSUM via scaled-identity
matmuls.
"""

import sys

for _p in ("/opt/trn_rl_repo",):
    if _p not in sys.path:
        sys.path.insert(0, _p)

import numpy as np
import ml_dtypes

import concourse.bass as bass
import concourse.bacc as bacc
import concourse.tile as tile
import concourse.mybir as mybir
from concourse.masks import make_identity

F32 = mybir.dt.float32
BF16 = mybir.dt.bfloat16
FP8 = mybir.dt.float8e4
I16 = mybir.dt.int16

AF = mybir.ActivationFunctionType
OP = mybir.AluOpType

NEG = -1e9


def mm_acc(nc, ps, lhsT3, rhs3, nsub, start, stop):
    """Accumulating matmul over `nsub` 128-contraction subtiles.
    lhsT3/rhs3: APs shaped [128, nsub, *]."""
    for s in range(nsub):
        nc.tensor.matmul(ps, lhsT3[:, s], rhs3[:, s],
                         start=(start and s == 0), stop=(stop and s == nsub - 1))


def mm_acc_multi(nc, pss, lhsT3, rhss, nsub, start, stop):
    """Like mm_acc but for several moving operands sharing the stationary
    subtiles: subtile-outer order so each lhsT subtile is loaded once."""
    for s in range(nsub):
        for i, (ps, rhs3) in enumerate(zip(pss, rhss)):
            nc.tensor.matmul(ps, lhsT3[:, s], rhs3[:, s],
                             start=(start and s == 0),
                             stop=(stop and s == nsub - 1))


class Cfg:
    def __init__(self, B=4, S=1024, D=1024, H=16, F=4096, L=4, V=32000, SPAN=512,
                 n_cores=8, act="gelu", no_cc=False):
        self.B, self.S, self.D, self.H, self.F, self.L, self.V, self.SPAN = (
            B, S, D, H, F, L, V, SPAN)
        self.n_cores = n_cores
        self.DH = D // H
        assert self.DH == 64
        self.DT = D // 128          # d tiles
        self.TT = S // 128          # token tiles
        self.NHL = H // 2           # heads per core
        self.DCL = self.NHL * self.DH   # local head-dim cols
        self.JT = self.DCL // 128   # local dcol tiles (2 heads per tile)
        self.FL = F // 2            # local ffn cols
        self.FT = self.FL // 128
        self.CH = min(512, S)       # token chunk
        self.NCH = S // self.CH
        self.CU = min(512, 2 * SPAN)
        self.NUC = (2 * SPAN) // self.CU
        self.SUB = min(4, self.DT)
        self.FSUB = min(4, self.FT)
        self.W = 2 * S              # extended table width
        self.scale = 1.0 / np.sqrt(3.0 * self.DH)
        self.act = act
        self.no_cc = no_cc


def build_nc(cfg):
    c = cfg
    nc = bacc.Bacc("TRN2", target_bir_lowering=False, debug=False,
                   num_devices=c.n_cores)

    def inp(name, shape, dt):
        return nc.dram_tensor(name, list(shape), dt, kind="ExternalInput")

    ids16 = inp("ids16", [128, c.S // 16], I16)
    tok_emb = inp("tok_emb", [c.V, c.D], F32)
    segsel = inp("segsel", [128, c.TT], F32)
    seg0rep = inp("seg0rep", [128, c.D], F32)
    segdrep = inp("segdrep", [128, c.D], F32)
    maskt = inp("maskt", [128, c.TT], F32)
    maskbias = inp("maskbias", [128, c.TT], F32)
    egrep = inp("egrep", [128, c.D], F32)
    ebrep = inp("ebrep", [128, c.D], F32)
    relT = inp("relT", [128, c.DT, 2 * c.SPAN], BF16)
    relTr = inp("relTr", [128, c.DT, 2 * c.SPAN], BF16)
    wqkv = inp("wqkv", [c.L, 128, c.DT, 3 * c.DCL], BF16)
    bqkv = inp("bqkv", [c.L, 128, 3 * c.JT], F32)
    bvrep = inp("bvrep", [c.L, 128, c.DCL], F32)
    wo = inp("wo", [c.L, 128, c.JT, c.D], BF16)
    bo2 = inp("bo2", [c.L, 1, c.D], BF16)
    w1 = inp("w1", [c.L, 128, c.DT, c.FL], BF16)
    b1 = inp("b1", [c.L, 128, c.FT], F32)
    w2 = inp("w2", [c.L, 128, c.FT, c.D], BF16)
    b22 = inp("b22", [c.L, 1, c.D], BF16)
    ln1g = inp("ln1g", [c.L, 128, c.DT], F32)
    ln1b = inp("ln1b", [c.L, 128, c.DT], F32)
    ln2g = inp("ln2g", [c.L, 128, c.DT], F32)
    ln2b = inp("ln2b", [c.L, 128, c.DT], F32)

    out_hT = nc.dram_tensor("out_hT", [128, c.DT, c.S], F32, kind="ExternalOutput")

    pairs = [[2 * i, 2 * i + 1] for i in range(c.n_cores // 2)]

    with tile.TileContext(nc) as tc:
        import contextlib
        est = contextlib.ExitStack()
        with est:
            const = est.enter_context(tc.tile_pool(name="const", bufs=1))
            resid = est.enter_context(tc.tile_pool(name="resid", bufs=1))
            dramp = est.enter_context(tc.tile_pool(name="dramp", bufs=4, space="DRAM"))
            wpool = est.enter_context(tc.tile_pool(name="wpool", bufs=4))

            identT = const.tile([128, 128], F32)
            make_identity(nc, identT[:])
            ident8 = const.tile([128, 128], FP8)
            nc.gpsimd.memset(ident8[:], 2.0 ** -8)
            nc.gpsimd.affine_select(
                out=ident8[:], in_=ident8[:], compare_op=OP.is_equal, fill=0.0,
                base=0, pattern=[[-1, 128]], channel_multiplier=1)
            ones1x64 = const.tile([1, 64], BF16)
            nc.vector.memset(ones1x64[:], 1.0)
            ones1x128 = const.tile([1, 128], F32)
            nc.vector.memset(ones1x128[:], 1.0)
            onesb = const.tile([128, c.SUB, 1], BF16)
            nc.vector.memset(onesb[:], 1.0)
            onesf = const.tile([128, c.SUB, 1], F32)
            nc.vector.memset(onesf[:], 1.0)
            onesrow = const.tile([1, c.CH], BF16)
            nc.vector.memset(onesrow[:], 1.0)
            eps1 = const.tile([1, 1], F32)
            nc.vector.memset(eps1[:], 1e-12)
            eps2 = const.tile([1, 1], F32)
            nc.vector.memset(eps2[:], float(c.D) ** 2 * 1e-12)
            invD_row = const.tile([1, 128], BF16)
            nc.vector.memset(invD_row[:], 1.0 / c.D)
            D_row = const.tile([1, 128], BF16)
            nc.vector.memset(D_row[:], float(c.D))
            mb_sb = const.tile([128, c.TT], F32)
            nc.sync.dma_start(mb_sb[:], maskbias.ap())

            hTbf = resid.tile([128, c.DT, c.S], BF16)

            # ---------------- embedding ----------------
            with (
                tc.tile_pool(name="embp", bufs=1) as embp,
                tc.tile_pool(name="embps", bufs=2, space="PSUM") as embps,
            ):
                ids_sb = embp.tile([128, c.S // 16], I16)
                nc.sync.dma_start(ids_sb[:], ids16.ap())
                gb = embp.tile([128, c.TT, c.D], F32)
                nc.gpsimd.dma_gather(
                    gb[:], tok_emb.ap(), ids_sb[:], num_idxs=c.S,
                    num_idxs_reg=c.S, elem_size=c.D)

                s0 = embp.tile([128, c.D], F32)
                nc.sync.dma_start(s0[:], seg0rep.ap())
                sd = embp.tile([128, c.D], F32)
                nc.sync.dma_start(sd[:], segdrep.ap())
                ssel = embp.tile([128, c.TT], F32)
                nc.sync.dma_start(ssel[:], segsel.ap())
                mt = embp.tile([128, c.TT], F32)
                nc.sync.dma_start(mt[:], maskt.ap())
                eg = embp.tile([128, c.D], F32)
                nc.sync.dma_start(eg[:], egrep.ap())
                eb = embp.tile([128, c.D], F32)
                nc.sync.dma_start(eb[:], ebrep.ap())

                # per-token-tile pipeline: token tile tt's transposes start
                # as soon as its stats are done instead of after the whole
                # batch of LN work.
                for tt in range(c.TT):
                    g1t = gb[:, tt]
                    nc.vector.tensor_tensor(
                        g1t, g1t, s0[:, None, :].to_broadcast((128, 1, c.D)),
                        OP.add)
                    nc.vector.scalar_tensor_tensor(
                        g1t, sd[:, None, :], ssel[:, tt:tt + 1], g1t,
                        OP.mult, OP.add)
                    mean = embp.tile([128, 1, 1], F32, tag=f"mean{tt}")
                    nc.vector.tensor_reduce(
                        mean[:], g1t, mybir.AxisListType.X, OP.add)
                    nc.vector.tensor_scalar_mul(mean[:], mean[:], 1.0 / c.D)
                    nc.vector.tensor_tensor(
                        g1t, g1t, mean[:].to_broadcast((128, 1, c.D)),
                        OP.subtract)
                    sq = embp.tile([128, 1, c.D], F32, tag=f"sq{tt}")
                    nc.scalar.square(sq[:], g1t)
                    var = embp.tile([128, 1, 1], F32, tag=f"var{tt}")
                    nc.vector.tensor_reduce(
                        var[:], sq[:], mybir.AxisListType.X, OP.add)
                    nc.vector.tensor_scalar(
                        var[:], var[:], 1.0 / c.D, 1e-12, OP.mult, OP.add)
                    rstd = embp.tile([128, 1, 1], F32, tag=f"rstd{tt}")
                    nc.vector.reciprocal(rstd[:], var[:])
                    nc.scalar.sqrt(rstd[:], rstd[:])
                    nc.vector.tensor_tensor(
                        g1t, g1t, rstd[:].to_broadcast((128, 1, c.D)), OP.mult)
                    nc.vector.tensor_tensor(
                        g1t, g1t, eg[:, None, :].to_broadcast((128, 1, c.D)),
                        OP.mult)
                    nc.vector.tensor_tensor(
                        g1t, g1t, eb[:, None, :].to_broadcast((128, 1, c.D)),
                        OP.add)
                    nc.vector.tensor_scalar_mul(g1t, g1t, mt[:, tt:tt + 1])

                    for dt in range(c.DT):
                        pst = embps.tile([128, 128], F32, tag="tp")
                        nc.tensor.transpose(
                            pst[:], gb[:, tt, dt * 128:(dt + 1) * 128], identT[:])
                        nc.vector.tensor_copy(
                            hTbf[:, dt, tt * 128:(tt + 1) * 128], pst[:])

            # ---------------- layers ----------------
            for l in range(c.L):
                layer(nc, tc, c, l, hTbf, mb_sb, dramp, wpool,
                      identT, ident8, ones1x64, ones1x128, onesb, eps1, onesrow,
                      eps2, invD_row, D_row, onesf,
                      wqkv, bqkv, bvrep, wo, bo2, w1, b1, w2, b22,
                      ln1g, ln1b, ln2g, ln2b, relT, relTr, pairs)

            with tc.tile_pool(name="outp", bufs=2) as outp:
                for dt in range(c.DT):
                    ot = outp.tile([128, c.S], F32, tag="o")
                    nc.scalar.copy(ot[:], hTbf[:, dt])
                    nc.sync.dma_start(out_hT.ap()[:, dt], ot[:])

    nc.compile()
    return nc


def layer(nc, tc, c, l, hTbf, mb_sb, dramp, wpool,
          identT, ident8, ones1x64, ones1x128, onesb, eps1, onesrow,
          eps2, invD_row, D_row, onesf,
          wqkv, bqkv, bvrep, wo, bo2, w1, b1, w2, b22,
          ln1g, ln1b, ln2g, ln2b, relT, relTr, pairs):
    S, D, CH, NCH = c.S, c.D, c.CH, c.NCH
    DT, TT, JT, FT, SUB = c.DT, c.TT, c.JT, c.FT, c.SUB

    with (
        tc.tile_pool(name=f"l{l}_ctx", bufs=1) as ctxp,
        tc.tile_pool(name=f"l{l}_misc", bufs=1) as miscp,
    ):
        ctxT = ctxp.tile([128, JT, S], BF16, name="ctxT")
        bq_sb = miscp.tile([128, 3 * JT], F32, name="bq_sb")
        nc.sync.dma_start(bq_sb[:], bqkv.ap()[l])
        bv_sb = miscp.tile([128, c.DCL], F32, name="bv_sb")
        nc.sync.dma_start(bv_sb[:], bvrep.ap()[l])
        bo_sb = miscp.tile([1, D], BF16, name="bo_sb")
        nc.sync.dma_start(bo_sb[:], bo2.ap()[l])
        b1_sb = miscp.tile([128, FT], F32, name="b1_sb")
        nc.sync.dma_start(b1_sb[:], b1.ap()[l])
        b2_sb = miscp.tile([1, D], BF16, name="b2_sb")
        nc.sync.dma_start(b2_sb[:], b22.ap()[l])

        attn_scope = tc.tile_pool(name=f"l{l}_qkv", bufs=1)
        qkvp = attn_scope.__enter__()
        qsT = qkvp.tile([128, JT, S], BF16, name="qsT")
        kT = qkvp.tile([128, JT, S], BF16, name="kT")
        v_sb = qkvp.tile([128, TT, c.NHL * 65], BF16, name="v_sb")
        PW = 2 * c.SPAN + 256   # pos tables padded 128 each side (clamp ext)
        poskr = qkvp.tile([128, JT, PW], BF16, name="poskr")
        posq = qkvp.tile([128, JT, PW], BF16, name="posq")
        # ---- phase A: pos tables, qkv/v projections ----
        with (
            tc.tile_pool(name=f"l{l}_rel", bufs=1) as relp,
            tc.tile_pool(name=f"l{l}_wqk", bufs=1) as wqkp,
            tc.tile_pool(name=f"l{l}_wv", bufs=1) as wvpool,
            tc.tile_pool(name=f"l{l}_pps", bufs=2, space="PSUM") as pps,
            tc.tile_pool(name=f"l{l}_ppsb", bufs=1, space="PSUM") as ppsb,
        ):
            # load all q/k weight col-tiles once (shared by pos + qkv proj)
            qkwt = []
            for proj in range(2):
                row = []
                for jt in range(JT):
                    wof = proj * c.DCL + jt * 128
                    wt = wqkp.tile([128, DT, 128], BF16, tag=f"wqk{proj}{jt}",
                                   name=f"wqk{proj}{jt}")
                    nc.sync.dma_start(wt[:], wqkv.ap()[l, :, :, wof:wof + 128])
                    row.append(wt)
                qkwt.append(row)

            # pos projections: pos_kT_rev from relTr/Wk, pos_qT(scaled) from
            # relT/Wq.  rel chunks loaded once per (table, chunk).
            for which, (dst, reltab, proj, pofs, scl) in enumerate(
                (
                    (poskr, relTr, 1, JT, 1.0),        # Wk part, bias bk
                    (posq, relT, 0, 0, c.scale),       # Wq part, bias bq*s
                )
            ):
                rts = []
                for uc in range(c.NUC):
                    rt = relp.tile([128, DT, c.CU], BF16, tag=f"rel{uc}")
                    nc.sync.dma_start(
                        rt[:], reltab.ap()[:, :, uc * c.CU:(uc + 1) * c.CU])
                    rts.append(rt)
                for jt in range(JT):
                    pss = [pps.tile([128, c.CU], F32, tag=f"pos{uc}", name=f"pos{uc}")
                           for uc in range(c.NUC)]
                    mm_acc_multi(nc, [p[:] for p in pss], qkwt[proj][jt][:],
                                 [r[:] for r in rts], DT, True, True)
                    for uc in range(c.NUC):
                        nc.scalar.activation(
                            dst[:, jt, 128 + uc * c.CU:128 + (uc + 1) * c.CU],
                            pss[uc][:],
                            AF.Identity, bias=bq_sb[:, pofs + jt:pofs + jt + 1],
                            scale=scl)
                nc.vector.tensor_scalar_mul(
                    dst[:, :, 0:128],
                    dst[:, :, 128:129].to_broadcast((128, JT, 128)), 1.0)
                nc.vector.tensor_scalar_mul(
                    dst[:, :, PW - 128:PW],
                    dst[:, :, PW - 129:PW - 128].to_broadcast((128, JT, 128)),
                    1.0)

            # qkv projections (feature-major q/k; token-major v),
            # chunk-outer so chunk 0's projections start right after LN2's
            # chunk 0 instead of waiting for the whole LN.
            for ch in range(NCH):
                for proj in range(2):  # 0=q, 1=k
                    dst = (qsT, kT)[proj]
                    scl = (c.scale, 1.0)[proj]
                    for jt in range(JT):
                        ps = ppsb.tile([128, CH], F32, tag=f"qkv{ch}",
                                       name=f"qkv{ch}")
                        mm_acc(nc, ps[:], qkwt[proj][jt][:],
                               hTbf[:, :, ch * CH:(ch + 1) * CH], DT,
                               True, True)
                        nc.scalar.activation(
                            dst[:, jt, ch * CH:(ch + 1) * CH], ps[:],
                            AF.Identity,
                            bias=bq_sb[:, proj * JT + jt:proj * JT + jt + 1],
                            scale=scl)
            # v: out[token, dv_loc]
            wtv = wvpool.tile([128, DT, c.DCL], BF16, tag="wv")
            nc.sync.dma_start(wtv[:], wqkv.ap()[l, :, :, 2 * c.DCL:3 * c.DCL])
            for tt in range(TT):
                ps = ppsb.tile([128, c.DCL], F32, tag="vproj")
                mm_acc(nc, ps[:], hTbf[:, :, tt * 128:(tt + 1) * 128],
                       wtv[:], DT, True, True)
                for hl in range(c.NHL):
                    nc.vector.tensor_tensor(
                        v_sb[:, tt, hl * 65:hl * 65 + 64],
                        ps[:, hl * 64:(hl + 1) * 64],
                        bv_sb[:, hl * 64:(hl + 1) * 64], OP.add)
            for hl in range(c.NHL):
                nc.vector.memset(v_sb[:, :, hl * 65 + 64:hl * 65 + 65], 1.0)

        # ---- phase B: per-head attention ----
        with (
            tc.tile_pool(name=f"l{l}_ct", bufs=4) as ctp,
            tc.tile_pool(name=f"l{l}_g1", bufs=3) as g1p,
            tc.tile_pool(name=f"l{l}_g2", bufs=3) as g2p,
            tc.tile_pool(name=f"l{l}_ex", bufs=2) as exp_,
            tc.tile_pool(name=f"l{l}_sc", bufs=2) as scp,
            tc.tile_pool(name=f"l{l}_bps", bufs=1, space="PSUM") as bps,
            tc.tile_pool(name=f"l{l}_bsc", bufs=2, space="PSUM") as bsc,
            tc.tile_pool(name=f"l{l}_bp2", bufs=1, space="PSUM") as bps2,
            tc.tile_pool(name=f"l{l}_bp3", bufs=1, space="PSUM") as bps3,
        ):
            MW = 2 * c.SPAN + 256     # widened mid (covers +-128 clamp)
            ML = S - c.SPAN - 128      # mid left col in the table

            def build_tables(hl):
                """Emit table-build matmuls + staging + DRAM writes + clamp
                pads for head hl.  Returns the two DRAM table tiles."""
                jt, rb = hl // 2, 64 * (hl % 2)
                qh = qsT[rb:rb + 64, jt]      # [64, S]
                kh = kT[rb:rb + 64, jt]
                pkh = poskr[rb:rb + 64, jt]   # [64, 2*SPAN]
                pqh = posq[rb:rb + 64, jt]

                cq_dr = dramp.tile([S, c.W], FP8, tag="cq", name=f"cq{l}_{hl}")
                ck_dr = dramp.tile([S, c.W], FP8, tag="ck", name=f"ck{l}_{hl}")
                # c2p table: rows q, mid cols = q_s . pos_k_rev; p2c: rows k
                for which, (dr, lh, rh) in enumerate(
                        ((cq_dr, qh, pkh), (ck_dr, kh, pqh))):
                    th = dr[:].tensor
                    base = dr[:].offset
                    for rt in range(TT):
                        st = ctp.tile([128, MW], FP8, tag="cstage")
                        # one 1024-wide psum over the real (unclamped) rel
                        # range; the 128-col clamp flanks are broadcast on
                        # the DVE afterwards.
                        ps = bps.tile([128, 1024], F32, tag=f"ctab{which}")
                        for co in range(2):
                            nc.tensor.matmul(
                                ps[:, co * 512:(co + 1) * 512],
                                lhsT=lh[:, rt * 128:(rt + 1) * 128],
                                rhs=rh[:, 128 + co * 512:128 + (co + 1) * 512],
                                start=True, stop=True)
                        if which == 0:
                            nc.scalar.activation(
                                st[:, 128:1152], ps[:], AF.Copy, scale=256.0)
                        else:
                            nc.vector.tensor_scalar_mul(
                                st[:, 128:1152], ps[:], 256.0)
                        nc.vector.tensor_scalar_mul(
                            st[:, 0:128],
                            st[:, 128:129].to_broadcast((128, 128)), 1.0)
                        nc.vector.tensor_scalar_mul(
                            st[:, 1152:1280],
                            st[:, 1151:1152].to_broadcast((128, 128)), 1.0)
                        dst = bass.AP(
                            th, base + (rt * 128) * c.W + ML,
                            [[c.W, 128], [1, MW]])
                        nc.sync.dma_start(dst, st[:])
                    # log-doubling clamp pads (row-constant regions)
                    pos, havew = ML, 128
                    while pos > 0:
                        w = min(pos, havew)
                        ldst = bass.AP(th, base + pos - w, [[c.W, S], [1, w]])
                        lsrc = bass.AP(th, base + pos, [[c.W, S], [1, w]])
                        nc.sync.dma_start(ldst, lsrc)
                        pos -= w
                        havew += w
                    pos, havew = ML + MW, 128
                    while pos < c.W:
                        w = min(c.W - pos, havew)
                        rdst = bass.AP(th, base + pos, [[c.W, S], [1, w]])
                        rsrc = bass.AP(th, base + pos - havew,
                                       [[c.W, S], [1, w]])
                        nc.sync.dma_start(rdst, rsrc)
                        pos += w
                        havew += w
                return cq_dr, ck_dr

            def attend(hl, cq_dr, ck_dr):
                """Scores + softmax + probs@v for head hl from its tables."""
                jt, rb = hl // 2, 64 * (hl % 2)
                qh = qsT[rb:rb + 64, jt]      # [64, S]
                kh = kT[rb:rb + 64, jt]

                g1 = g1p.tile([128, TT, S], FP8, tag="g1")
                thq = cq_dr[:].tensor
                bq_ = cq_dr[:].offset
                for qt in range(TT):
                    src = bass.AP(thq, bq_ + (c.W - 1) * (qt * 128) + S - 1,
                                  [[c.W - 1, 128], [1, S]])
                    nc.sync.dma_start(g1[:, qt], src)

                ex = exp_.tile([128, TT, S], BF16, tag="ex")
                thk = ck_dr[:].tensor
                bk_ = ck_dr[:].offset
                for kt in range(TT):
                    g2 = g2p.tile([128, S], FP8, tag="g2", name=f"g2_{kt}")
                    src = bass.AP(thk, bk_ + (c.W - 1) * (kt * 128) + S,
                                  [[c.W - 1, 128], [1, S]])
                    nc.sync.dma_start(g2[:], src)
                    for ch in range(NCH):
                        ps = bsc.tile([128, CH], F32, tag="scores")
                        nc.tensor.matmul(
                            ps[:], lhsT=kh[:, kt * 128:(kt + 1) * 128],
                            rhs=qh[:, ch * CH:(ch + 1) * CH],
                            start=True, stop=False)
                        nc.tensor.matmul(
                            ps[:], lhsT=ident8[:],
                            rhs=g2[:, ch * CH:(ch + 1) * CH],
                            start=False, stop=False)
                        nq = CH // 128
                        for qi in range(nq):
                            qt = ch * nq + qi
                            nc.tensor.matmul(
                                ps[:, qi * 128:(qi + 1) * 128],
                                lhsT=g1[:, qt, kt * 128:(kt + 1) * 128],
                                rhs=ident8[:],
                                start=False, stop=True,
                                skip_group_check=(qi != nq - 1))
                        nc.scalar.activation(
                            ex[:, kt, ch * CH:(ch + 1) * CH], ps[:], AF.Exp,
                            bias=mb_sb[:, kt:kt + 1], scale=1.0)

                for ch in range(NCH):
                    pv = bps2.tile([65, CH], F32, tag="pv")
                    for kt in range(TT):
                        nc.tensor.matmul(
                            pv[:], lhsT=v_sb[:, kt, hl * 65:hl * 65 + 65],
                            rhs=ex[:, kt, ch * CH:(ch + 1) * CH],
                            start=(kt == 0), stop=(kt == TT - 1))
                    rec = scp.tile([1, CH], BF16, tag="rec")
                    with nc.allow_low_precision(reason="softmax denom bf16"):
                        nc.vector.reciprocal(rec[:], pv[64:65, :])
                    pb = bps3.tile([64, CH], F32, tag="recb")
                    nc.tensor.matmul(pb[:], lhsT=ones1x64[:], rhs=rec[:],
                                     start=True, stop=True)
                    rb_sb = scp.tile([64, CH], F32, tag="recbs")
                    nc.scalar.copy(rb_sb[:], pb[:])
                    nc.vector.tensor_tensor(
                        ctxT[rb:rb + 64, jt, ch * CH:(ch + 1) * CH],
                        pv[0:64, :], rb_sb[:], OP.mult)

            # software-pipeline heads: the PE queue is in-order, so head
            # h's score matmuls must not sit at the queue head while h's
            # table DRAM roundtrip is still in flight — keep LOOKAHEAD
            # heads of table builds queued ahead.
            LOOKAHEAD = 3
            built = {}
            for j in range(min(LOOKAHEAD + 1, c.NHL)):
                built[j] = build_tables(j)
            for hl in range(c.NHL):
                attend(hl, *built.pop(hl))
                nxt = hl + LOOKAHEAD + 1
                if nxt < c.NHL:
                    built[nxt] = build_tables(nxt)

        attn_scope.__exit__(None, None, None)   # free qsT/kT/v/pos SBUF

        # ---- phase C: Wo + AR + LN1 ----
        # per-token-chunk AR tensors: chunk 0's reduce+LN runs while chunk 1
        # is still accumulating
        ar1 = [dramp.tile([128, DT, CH], BF16, tag=f"arin{ch}",
                          name=f"ar1i_{l}_{ch}") for ch in range(NCH)]
        ar1o = [dramp.tile([128, DT, CH], BF16, tag=f"arout{ch}",
                           name=f"ar1o_{l}_{ch}") for ch in range(NCH)]
        with (
            tc.tile_pool(name=f"l{l}_wops", bufs=1, space="PSUM") as wops,
            tc.tile_pool(name=f"l{l}_wost", bufs=3) as wost,
            tc.tile_pool(name=f"l{l}_wo", bufs=1) as wopool,
        ):
            wos = wopool.tile([128, JT, D], BF16, tag="wo")
            nc.sync.dma_start(wos[:], wo.ap()[l])
            # chunk-outer so chunk 0's AllReduce fires while chunk 1 runs
            for ch in range(NCH):
                for dt in range(DT):
                    ps = wops.tile([128, CH], F32, tag=f"wo{ch}",
                                   name=f"wo{ch}")
                    mm_acc(nc, ps[:], wos[:, :, dt * 128:(dt + 1) * 128],
                           ctxT[:, :, ch * CH:(ch + 1) * CH], JT, True, False)
                    nc.tensor.matmul(
                        ps[:], lhsT=bo_sb[:, dt * 128:(dt + 1) * 128],
                        rhs=onesrow[:], start=False, stop=True)
                    st = wost.tile([128, CH], BF16, tag="wost")
                    nc.vector.scalar_tensor_tensor(
                        st[:], hTbf[:, dt, ch * CH:(ch + 1) * CH], 0.5,
                        ps[:], OP.mult, OP.add)
                    nc.sync.dma_start(ar1[ch][:, dt], st[:])
                if c.n_cores == 1 or c.no_cc:
                    nc.sync.dma_start(ar1o[ch][:], ar1[ch][:])
                else:
                    nc.gpsimd.collective_compute(
                        "AllReduce", OP.add, replica_groups=pairs,
                        ins=[ar1[ch].opt()], outs=[ar1o[ch].opt()])
        _ln(nc, tc, c, l, ar1o, hTbf, ln1g, ln1b, onesb, eps2,
            invD_row, D_row, onesf)

        # ---- phase D: FFN + AR + LN2 ----
        ar2 = [dramp.tile([128, DT, CH], BF16, tag=f"arin{ch}",
                          name=f"ar2i_{l}_{ch}") for ch in range(NCH)]
        ar2o = [dramp.tile([128, DT, CH], BF16, tag=f"arout{ch}",
                           name=f"ar2o_{l}_{ch}") for ch in range(NCH)]
        with (
            tc.tile_pool(name=f"l{l}_gt", bufs=2) as gtp,
            tc.tile_pool(name=f"l{l}_w1", bufs=1) as w1pool,
            tc.tile_pool(name=f"l{l}_w2", bufs=1) as w2pool,
            tc.tile_pool(name=f"l{l}_f1ps", bufs=1, space="PSUM") as f1ps,
            tc.tile_pool(name=f"l{l}_f2ps", bufs=1, space="PSUM") as f2ps,
            tc.tile_pool(name=f"l{l}_fst", bufs=3) as fst,
        ):
            w1t = w1pool.tile([128, DT, c.FL], BF16, tag="w1")
            nc.sync.dma_start(w1t[:], w1.ap()[l])
            w2t = w2pool.tile([128, FT, D], BF16, tag="w2")
            nc.sync.dma_start(w2t[:], w2.ap()[l])
            gts = [gtp.tile([128, FT, CH], BF16, tag="gt", name=f"gt{ch}")
                   for ch in range(NCH)]
            # chunk-outer: chunk 0 flows W1 -> W2 -> AR while chunk 1
            # computes, so the AllReduce latency overlaps compute.
            for ch in range(NCH):
                hchunk = hTbf[:, :, ch * CH:(ch + 1) * CH]
                for ft in range(FT):
                    ps = f1ps.tile([128, CH], F32, tag=f"f1{ch}",
                                   name=f"f1{ch}")
                    mm_acc(nc, ps[:], w1t[:, :, ft * 128:(ft + 1) * 128],
                           hchunk, DT, True, True)
                    nc.scalar.activation(
                        gts[ch][:, ft], ps[:],
                        AF.Gelu if c.act == "gelu" else AF.Relu,
                        bias=b1_sb[:, ft:ft + 1], scale=1.0)
                for dt in range(DT):
                    ps = f2ps.tile([128, CH], F32, tag=f"f2{ch}",
                                   name=f"f2{ch}")
                    mm_acc(nc, ps[:], w2t[:, :, dt * 128:(dt + 1) * 128],
                           gts[ch][:], FT, True, False)
                    nc.tensor.matmul(
                        ps[:], lhsT=b2_sb[:, dt * 128:(dt + 1) * 128],
                        rhs=onesrow[:], start=False, stop=True)
                    st = fst.tile([128, CH], BF16, tag="fst")
                    nc.vector.scalar_tensor_tensor(
                        st[:], hTbf[:, dt, ch * CH:(ch + 1) * CH], 0.5,
                        ps[:], OP.mult, OP.add)
                    nc.sync.dma_start(ar2[ch][:, dt], st[:])
                if c.n_cores == 1 or c.no_cc:
                    nc.sync.dma_start(ar2o[ch][:], ar2[ch][:])
                else:
                    nc.gpsimd.collective_compute(
                        "AllReduce", OP.add, replica_groups=pairs,
                        ins=[ar2[ch].opt()], outs=[ar2o[ch].opt()])
        _ln(nc, tc, c, l, ar2o, hTbf, ln2g, ln2b, onesb, eps2,
            invD_row, D_row, onesf)


def _ln(nc, tc, c, l, x_drs, hTbf, g_in, b_in, onesb, eps2,
        invD_row, D_row, onesf):
    """Feature-major layernorm over partitions: x in per-chunk DRAM tiles
    [128, DT, CH] bf16 -> hTbf.  Single pass over x: tiles are kept in SBUF
    between the stats accumulation and the normalize step.
    rstd computed as D/sqrt(D*s1 - s0^2 + D^2*eps)."""
    S, CH, NCH, DT, SUB = c.S, c.CH, c.NCH, c.DT, c.SUB
    with (
        tc.tile_pool(name=f"ln{l}", bufs=2) as lp,
        tc.tile_pool(name=f"ln{l}s", bufs=1) as lps,
        tc.tile_pool(name=f"ln{l}ps", bufs=1, space="PSUM") as pps,
        tc.tile_pool(name=f"ln{l}pb", bufs=2, space="PSUM") as pbs,
    ):
        g_sb = lps.tile([128, DT], F32, tag="g")
        nc.sync.dma_start(g_sb[:], g_in.ap()[l])
        b_sb = lps.tile([128, DT], F32, tag="b")
        nc.sync.dma_start(b_sb[:], b_in.ap()[l])

        stats0 = pps.tile([1, S], F32, tag="stats0")
        stats1 = pps.tile([1, S], F32, tag="stats1")
        ngr = DT // SUB
        s0 = lps.tile([1, S], F32, tag="s0")
        s0b = lps.tile([1, S], BF16, tag="s0b")
        s1 = lps.tile([1, S], F32, tag="s1")
        u = lps.tile([1, S], F32, tag="u")
        rp = lps.tile([1, S], BF16, tag="rp")
        mu_b = lps.tile([128, S], F32, tag="mub")
        rs_b = lps.tile([128, S], F32, tag="rsb")
        # fully per-chunk: stats(ch) -> rstd(ch) -> bcast(ch) ->
        # normalize(ch) before chunk ch+1's stats, so chunk 0's output
        # unblocks downstream consumers while chunk 1 still reduces.
        for ch in range(NCH):
            xts = {}
            for g in range(ngr):
                xt = lps.tile([128, SUB, CH], BF16, tag=f"x{ch}_{g}")
                xts[g] = xt
                nc.sync.dma_start(
                    xt[:], x_drs[ch][:, g * SUB:(g + 1) * SUB])
                x2 = lp.tile([128, SUB, CH], BF16, tag="x2")
                nc.scalar.square(x2[:], xt[:])
                for s in range(SUB):
                    nc.tensor.matmul(
                        stats0[:, ch * CH:(ch + 1) * CH], lhsT=onesb[:, s],
                        rhs=xt[:, s], start=(g == 0 and s == 0),
                        stop=(g == ngr - 1 and s == SUB - 1))
                    nc.tensor.matmul(
                        stats1[:, ch * CH:(ch + 1) * CH], lhsT=onesb[:, s],
                        rhs=x2[:, s], start=(g == 0 and s == 0),
                        stop=(g == ngr - 1 and s == SUB - 1))
            cs = slice(ch * CH, (ch + 1) * CH)
            nc.scalar.copy(s0[:, cs], stats0[:, cs])
            nc.scalar.copy(s1[:, cs], stats1[:, cs])
            nc.vector.tensor_copy(s0b[:, cs], s0[:, cs])
            nc.vector.tensor_tensor(u[:, cs], s0[:, cs], s0[:, cs], OP.mult)
            nc.vector.scalar_tensor_tensor(
                u[:, cs], s1[:, cs], float(c.D), u[:, cs], OP.mult, OP.subtract)
            nc.scalar.activation(u[:, cs], u[:, cs], AF.Sqrt, bias=eps2[:],
                                 scale=1.0)
            with nc.allow_low_precision(reason="rstd bf16 broadcast"):
                nc.vector.reciprocal(rp[:, cs], u[:, cs])
            pm = pbs.tile([128, CH], F32, tag="bc")
            nc.tensor.matmul(pm[:], lhsT=invD_row[:], rhs=s0b[0:1, cs],
                             start=True, stop=True)
            nc.scalar.copy(mu_b[:, cs], pm[:])
            pr = pbs.tile([128, CH], F32, tag="bc")
            nc.tensor.matmul(pr[:], lhsT=D_row[:], rhs=rp[0:1, cs],
                             start=True, stop=True)
            nc.scalar.copy(rs_b[:, cs], pr[:])

            for g in range(ngr):
                xt = xts[g]
                xn = lp.tile([128, SUB, CH], F32, tag="xn")
                mub = mu_b[:, None, ch * CH:(ch + 1) * CH].to_broadcast(
                    (128, SUB, CH))
                nc.vector.tensor_tensor(xn[:], xt[:], mub, OP.subtract)
                rsb = rs_b[:, None, ch * CH:(ch + 1) * CH].to_broadcast(
                    (128, SUB, CH))
                nc.vector.tensor_tensor(xn[:], xn[:], rsb, OP.mult)
                for i in range(SUB):
                    dt = g * SUB + i
                    nc.scalar.activation(
                        hTbf[:, dt, ch * CH:(ch + 1) * CH], xn[:, i],
                        AF.Identity, bias=b_sb[:, dt:dt + 1],
                        scale=g_sb[:, dt:dt + 1])


# ---------------------------------------------------------------------------
# host side
# ---------------------------------------------------------------------------

def host_prep(c, inputs):
    """Build per-core in_maps from full inputs."""
    bf = ml_dtypes.bfloat16
    f32 = np.float32
    ii = {k: np.asarray(v) for k, v in inputs.items()}
    S, D, L = c.S, c.D, c.L

    def tokmaj(vec):  # [S] -> [128, TT]   t = tt*128 + p
        return np.ascontiguousarray(vec.reshape(c.TT, 128).T)

    def dpart(vec):  # [D] -> [128, DT]
        return np.ascontiguousarray(vec.reshape(c.DT, 128).T)

    rel = ii["rel_emb"].astype(f32)  # [2*SPAN, D]
    relT = np.ascontiguousarray(
        rel.T.reshape(c.DT, 128, 2 * c.SPAN).transpose(1, 0, 2)).astype(bf)
    relr = rel[::-1]
    relTr = np.ascontiguousarray(
        relr.T.reshape(c.DT, 128, 2 * c.SPAN).transpose(1, 0, 2)).astype(bf)

    in_maps = []
    for core in range(c.n_cores):
        b, half = core // 2, core % 2
        colr = slice(half * c.DCL, (half + 1) * c.DCL)
        fcol = slice(half * c.FL, (half + 1) * c.FL)

        ids = ii["input_ids"][b].astype(np.int64)
        w = np.zeros((16, S // 16), np.int16)
        for i in range(S):
            w[i % 16, i // 16] = ids[i]
        ids16 = np.tile(w, (8, 1))

        seg = ii["segment_ids"][b].astype(f32)
        mask = ii["attention_mask"][b].astype(f32)

        wq = ii["Wq"][:, :, colr].astype(f32)
        wk = ii["Wk"][:, :, colr].astype(f32)
        wv = ii["Wv"][:, :, colr].astype(f32)
        wqkv = np.concatenate([wq, wk, wv], axis=2)  # [L, D, 3*DCL]
        wqkv = wqkv.reshape(L, c.DT, 128, 3 * c.DCL).transpose(0, 2, 1, 3)

        bq = ii["bq"][:, colr].astype(f32) * c.scale
        bk = ii["bk"][:, colr].astype(f32)
        bv = ii["bv"][:, colr].astype(f32)
        bqkv = np.concatenate(
            [bq.reshape(L, c.JT, 128).transpose(0, 2, 1),
             bk.reshape(L, c.JT, 128).transpose(0, 2, 1),
             np.zeros((L, 128, c.JT), f32)], axis=2)
        bvrep = np.broadcast_to(bv[:, None, :], (L, 128, c.DCL))

        wo_ = ii["Wo"][:, colr, :].astype(f32)
        wo_ = wo_.reshape(L, c.JT, 128, D).transpose(0, 2, 1, 3)
        bo2 = (ii["bo"].astype(f32) / 2.0)[:, None, :]

        w1_ = ii["W1"][:, :, fcol].astype(f32)
        w1_ = w1_.reshape(L, c.DT, 128, c.FL).transpose(0, 2, 1, 3)
        b1_ = ii["b1"][:, fcol].astype(f32).reshape(L, c.FT, 128).transpose(0, 2, 1)
        w2_ = ii["W2"][:, fcol, :].astype(f32)
        w2_ = w2_.reshape(L, c.FT, 128, D).transpose(0, 2, 1, 3)
        b22 = (ii["b2"].astype(f32) / 2.0)[:, None, :]

        m = {
            "ids16": ids16,
            "tok_emb": ii["tok_emb"].astype(f32),
            "segsel": tokmaj(seg),
            "seg0rep": np.broadcast_to(
                ii["seg_emb"][0].astype(f32), (128, D)).copy(),
            "segdrep": np.broadcast_to(
                (ii["seg_emb"][1] - ii["seg_emb"][0]).astype(f32),
                (128, D)).copy(),
            "maskt": tokmaj(mask),
            "maskbias": tokmaj(NEG * (1.0 - mask)),
            "egrep": np.broadcast_to(
                ii["emb_ln_g"].astype(f32), (128, D)).copy(),
            "ebrep": np.broadcast_to(
                ii["emb_ln_b"].astype(f32), (128, D)).copy(),
            "relT": relT,
            "relTr": relTr,
            "wqkv": wqkv.astype(bf),
            "bqkv": np.ascontiguousarray(bqkv),
            "bvrep": np.ascontiguousarray(bvrep),
            "wo": wo_.astype(bf),
            "bo2": bo2.astype(bf),
            "w1": w1_.astype(bf),
            "b1": np.ascontiguousarray(b1_),
            "w2": w2_.astype(bf),
            "b22": b22.astype(bf),
            "ln1g": ii["ln1_g"].astype(f32).reshape(
                L, c.DT, 128).transpose(0, 2, 1),
            "ln1b": ii["ln1_b"].astype(f32).reshape(
                L, c.DT, 128).transpose(0, 2, 1),
            "ln2g": ii["ln2_g"].astype(f32).reshape(
                L, c.DT, 128).transpose(0, 2, 1),
            "ln2b": ii["ln2_b"].astype(f32).reshape(
                L, c.DT, 128).transpose(0, 2, 1),
        }
        m = {k: np.ascontiguousarray(v) for k, v in m.items()}
        in_maps.append(m)
    return in_maps


def assemble(c, results):
    """results[core]["out_hT"] [128, DT, S] -> [B, S, D] fp32."""
    out = np.zeros((c.B, c.S, c.D), np.float32)
    for b in range(c.B):
        hT = results[2 * b]["out_hT"]  # [128, DT, S]
        out[b] = hT.transpose(2, 1, 0).reshape(c.S, c.D)
    return out


_nc_cache = {}


def _get_nc(c):
    key = (c.B, c.S, c.D, c.H, c.F, c.L, c.V, c.SPAN, c.n_cores)
    if key not in _nc_cache:
        _nc_cache[key] = build_nc(c)
    return _nc_cache[key]


def kernel(**inputs):
    from concourse import bass_utils
    c = Cfg()
    nc = _get_nc(c)
    in_maps = host_prep(c, inputs)
    res = bass_utils.run_bass_kernel_spmd(
        nc, in_maps, core_ids=list(range(c.n_cores)))
    return assemble(c, res.results)



# revision 61
# speedup vs baseline: 1.1139x; 1.1139x over previous
"""DeBERTa-bare Trainium2 Bass kernel.

Topology: 8 NeuronCores = 4 data-parallel pairs (one batch element each) x
2-way tensor parallel (heads + FFN split) with pairwise AllReduce.

Everything on-chip runs feature-major ("transposed"): h is kept as
hT[d, token].  The DeBERTa disentangled-attention gathers
(take_along_axis over relative positions) are realized as affine "skew"
access-pattern DMA reads from DRAM-resident, clamp-extended c2p/p2c tables
(fp8, x256 scaled), injected into the score PSUM via scaled-identity
matmuls.
"""

import sys

for _p in ("/opt/trn_rl_repo",):
    if _p not in sys.path:
        sys.path.insert(0, _p)

import numpy as np
import ml_dtypes

import concourse.bass as bass
import concourse.bacc as bacc
import concourse.tile as tile
import concourse.mybir as mybir
from concourse.masks import make_identity

F32 = mybir.dt.float32
BF16 = mybir.dt.bfloat16
FP8 = mybir.dt.float8e4
I16 = mybir.dt.int16

AF = mybir.ActivationFunctionType
OP = mybir.AluOpType

NEG = -1e9


def mm_acc(nc, ps, lhsT3, rhs3, nsub, start, stop):
    """Accumulating matmul over `nsub` 128-contraction subtiles.
    lhsT3/rhs3: APs shaped [128, nsub, *]."""
    for s in range(nsub):
        nc.tensor.matmul(ps, lhsT3[:, s], rhs3[:, s],
                         start=(start and s == 0), stop=(stop and s == nsub - 1))


def mm_acc_multi(nc, pss, lhsT3, rhss, nsub, start, stop):
    """Like mm_acc but for several moving operands sharing the stationary
    subtiles: subtile-outer order so each lhsT subtile is loaded once."""
    for s in range(nsub):
        for i, (ps, rhs3) in enumerate(zip(pss, rhss)):
            nc.tensor.matmul(ps, lhsT3[:, s], rhs3[:, s],
                             start=(start and s == 0),
                             stop=(stop and s == nsub - 1))


class Cfg:
    def __init__(self, B=4, S=1024, D=1024, H=16, F=4096, L=4, V=32000, SPAN=512,
                 n_cores=8, act="gelu", no_cc=False):
        self.B, self.S, self.D, self.H, self.F, self.L, self.V, self.SPAN = (
            B, S, D, H, F, L, V, SPAN)
        self.n_cores = n_cores
        self.DH = D // H
        assert self.DH == 64
        self.DT = D // 128          # d tiles
        self.TT = S // 128          # token tiles
        self.NHL = H // 2           # heads per core
        self.DCL = self.NHL * self.DH   # local head-dim cols
        self.JT = self.DCL // 128   # local dcol tiles (2 heads per tile)
        self.FL = F // 2            # local ffn cols
        self.FT = self.FL // 128
        self.CH = min(512, S)       # token chunk
        self.NCH = S // self.CH
        self.CU = min(512, 2 * SPAN)
        self.NUC = (2 * SPAN) // self.CU
        self.SUB = min(4, self.DT)
        self.FSUB = min(4, self.FT)
        self.W = 2 * S              # extended table width
        self.scale = 1.0 / np.sqrt(3.0 * self.DH)
        self.act = act
        self.no_cc = no_cc


def build_nc(cfg):
    c = cfg
    nc = bacc.Bacc("TRN2", target_bir_lowering=False, debug=False,
                   num_devices=c.n_cores)

    def inp(name, shape, dt):
        return nc.dram_tensor(name, list(shape), dt, kind="ExternalInput")

    ids16 = inp("ids16", [128, c.S // 16], I16)
    tok_emb = inp("tok_emb", [c.V, c.D], F32)
    segsel = inp("segsel", [128, c.TT], F32)
    seg0rep = inp("seg0rep", [128, c.D], F32)
    segdrep = inp("segdrep", [128, c.D], F32)
    maskt = inp("maskt", [128, c.TT], F32)
    maskbias = inp("maskbias", [128, c.TT], F32)
    egrep = inp("egrep", [128, c.D], F32)
    ebrep = inp("ebrep", [128, c.D], F32)
    relT = inp("relT", [128, c.DT, 2 * c.SPAN], BF16)
    relTr = inp("relTr", [128, c.DT, 2 * c.SPAN], BF16)
    wqkv = inp("wqkv", [c.L, 128, c.DT, 3 * c.DCL], BF16)
    bqkv = inp("bqkv", [c.L, 128, 3 * c.JT], F32)
    bvrep = inp("bvrep", [c.L, 128, c.DCL], F32)
    wo = inp("wo", [c.L, 128, c.JT, c.D], BF16)
    bo2 = inp("bo2", [c.L, 1, c.D], BF16)
    w1 = inp("w1", [c.L, 128, c.DT, c.FL], BF16)
    b1 = inp("b1", [c.L, 128, c.FT], F32)
    w2 = inp("w2", [c.L, 128, c.FT, c.D], BF16)
    b22 = inp("b22", [c.L, 1, c.D], BF16)
    ln1g = inp("ln1g", [c.L, 128, c.DT], F32)
    ln1b = inp("ln1b", [c.L, 128, c.DT], F32)
    ln2g = inp("ln2g", [c.L, 128, c.DT], F32)
    ln2b = inp("ln2b", [c.L, 128, c.DT], F32)

    out_hT = nc.dram_tensor("out_hT", [128, c.DT, c.S], F32, kind="ExternalOutput")

    pairs = [[2 * i, 2 * i + 1] for i in range(c.n_cores // 2)]

    with tile.TileContext(nc) as tc:
        import contextlib
        est = contextlib.ExitStack()
        with est:
            const = est.enter_context(tc.tile_pool(name="const", bufs=1))
            resid = est.enter_context(tc.tile_pool(name="resid", bufs=1))
            dramp = est.enter_context(tc.tile_pool(name="dramp", bufs=3, space="DRAM"))
            wpool = est.enter_context(tc.tile_pool(name="wpool", bufs=4))

            identT = const.tile([128, 128], F32)
            make_identity(nc, identT[:])
            ident8 = const.tile([128, 128], FP8)
            nc.gpsimd.memset(ident8[:], 2.0 ** -8)
            nc.gpsimd.affine_select(
                out=ident8[:], in_=ident8[:], compare_op=OP.is_equal, fill=0.0,
                base=0, pattern=[[-1, 128]], channel_multiplier=1)
            ones1x64 = const.tile([1, 64], BF16)
            nc.vector.memset(ones1x64[:], 1.0)
            ones1x128 = const.tile([1, 128], F32)
            nc.vector.memset(ones1x128[:], 1.0)
            onesb = const.tile([128, c.SUB, 1], BF16)
            nc.vector.memset(onesb[:], 1.0)
            onesf = const.tile([128, c.SUB, 1], F32)
            nc.vector.memset(onesf[:], 1.0)
            onesrow = const.tile([1, c.CH], BF16)
            nc.vector.memset(onesrow[:], 1.0)
            eps1 = const.tile([1, 1], F32)
            nc.vector.memset(eps1[:], 1e-12)
            eps2 = const.tile([1, 1], F32)
            nc.vector.memset(eps2[:], float(c.D) ** 2 * 1e-12)
            invD_row = const.tile([1, 128], BF16)
            nc.vector.memset(invD_row[:], 1.0 / c.D)
            D_row = const.tile([1, 128], BF16)
            nc.vector.memset(D_row[:], float(c.D))
            mb_sb = const.tile([128, c.TT], F32)
            nc.sync.dma_start(mb_sb[:], maskbias.ap())

            hTbf = resid.tile([128, c.DT, c.S], BF16)

            # ---------------- embedding ----------------
            with (
                tc.tile_pool(name="embp", bufs=1) as embp,
                tc.tile_pool(name="embps", bufs=2, space="PSUM") as embps,
            ):
                ids_sb = embp.tile([128, c.S // 16], I16)
                nc.sync.dma_start(ids_sb[:], ids16.ap())
                gb = embp.tile([128, c.TT, c.D], F32)
                nc.gpsimd.dma_gather(
                    gb[:], tok_emb.ap(), ids_sb[:], num_idxs=c.S,
                    num_idxs_reg=c.S, elem_size=c.D)

                s0 = embp.tile([128, c.D], F32)
                nc.sync.dma_start(s0[:], seg0rep.ap())
                sd = embp.tile([128, c.D], F32)
                nc.sync.dma_start(sd[:], segdrep.ap())
                ssel = embp.tile([128, c.TT], F32)
                nc.sync.dma_start(ssel[:], segsel.ap())
                mt = embp.tile([128, c.TT], F32)
                nc.sync.dma_start(mt[:], maskt.ap())
                eg = embp.tile([128, c.D], F32)
                nc.sync.dma_start(eg[:], egrep.ap())
                eb = embp.tile([128, c.D], F32)
                nc.sync.dma_start(eb[:], ebrep.ap())

                # per-token-tile pipeline: token tile tt's transposes start
                # as soon as its stats are done instead of after the whole
                # batch of LN work.
                for tt in range(c.TT):
                    g1t = gb[:, tt]
                    nc.vector.tensor_tensor(
                        g1t, g1t, s0[:, None, :].to_broadcast((128, 1, c.D)),
                        OP.add)
                    nc.vector.scalar_tensor_tensor(
                        g1t, sd[:, None, :], ssel[:, tt:tt + 1], g1t,
                        OP.mult, OP.add)
                    mean = embp.tile([128, 1, 1], F32, tag=f"mean{tt}")
                    nc.vector.tensor_reduce(
                        mean[:], g1t, mybir.AxisListType.X, OP.add)
                    nc.vector.tensor_scalar_mul(mean[:], mean[:], 1.0 / c.D)
                    nc.vector.tensor_tensor(
                        g1t, g1t, mean[:].to_broadcast((128, 1, c.D)),
                        OP.subtract)
                    sq = embp.tile([128, 1, c.D], F32, tag=f"sq{tt}")
                    nc.scalar.square(sq[:], g1t)
                    var = embp.tile([128, 1, 1], F32, tag=f"var{tt}")
                    nc.vector.tensor_reduce(
                        var[:], sq[:], mybir.AxisListType.X, OP.add)
                    nc.vector.tensor_scalar(
                        var[:], var[:], 1.0 / c.D, 1e-12, OP.mult, OP.add)
                    rstd = embp.tile([128, 1, 1], F32, tag=f"rstd{tt}")
                    nc.vector.reciprocal(rstd[:], var[:])
                    nc.scalar.sqrt(rstd[:], rstd[:])
                    nc.vector.tensor_tensor(
                        g1t, g1t, rstd[:].to_broadcast((128, 1, c.D)), OP.mult)
                    nc.vector.tensor_tensor(
                        g1t, g1t, eg[:, None, :].to_broadcast((128, 1, c.D)),
                        OP.mult)
                    nc.vector.tensor_tensor(
                        g1t, g1t, eb[:, None, :].to_broadcast((128, 1, c.D)),
                        OP.add)
                    nc.vector.tensor_scalar_mul(g1t, g1t, mt[:, tt:tt + 1])

                    for dt in range(c.DT):
                        pst = embps.tile([128, 128], F32, tag="tp")
                        nc.tensor.transpose(
                            pst[:], gb[:, tt, dt * 128:(dt + 1) * 128], identT[:])
                        nc.vector.tensor_copy(
                            hTbf[:, dt, tt * 128:(tt + 1) * 128], pst[:])

            # ---------------- layers ----------------
            for l in range(c.L):
                layer(nc, tc, c, l, hTbf, mb_sb, dramp, wpool,
                      identT, ident8, ones1x64, ones1x128, onesb, eps1, onesrow,
                      eps2, invD_row, D_row, onesf,
                      wqkv, bqkv, bvrep, wo, bo2, w1, b1, w2, b22,
                      ln1g, ln1b, ln2g, ln2b, relT, relTr, pairs)

            with tc.tile_pool(name="outp", bufs=2) as outp:
                for dt in range(c.DT):
                    ot = outp.tile([128, c.S], F32, tag="o")
                    nc.scalar.copy(ot[:], hTbf[:, dt])
                    nc.sync.dma_start(out_hT.ap()[:, dt], ot[:])

    nc.compile()
    return nc


def layer(nc, tc, c, l, hTbf, mb_sb, dramp, wpool,
          identT, ident8, ones1x64, ones1x128, onesb, eps1, onesrow,
          eps2, invD_row, D_row, onesf,
          wqkv, bqkv, bvrep, wo, bo2, w1, b1, w2, b22,
          ln1g, ln1b, ln2g, ln2b, relT, relTr, pairs):
    S, D, CH, NCH = c.S, c.D, c.CH, c.NCH
    DT, TT, JT, FT, SUB = c.DT, c.TT, c.JT, c.FT, c.SUB

    with (
        tc.tile_pool(name=f"l{l}_ctx", bufs=1) as ctxp,
        tc.tile_pool(name=f"l{l}_misc", bufs=1) as miscp,
    ):
        ctxT = ctxp.tile([128, JT, S], BF16, name="ctxT")
        bq_sb = miscp.tile([128, 3 * JT], F32, name="bq_sb")
        nc.sync.dma_start(bq_sb[:], bqkv.ap()[l])
        bv_sb = miscp.tile([128, c.DCL], F32, name="bv_sb")
        nc.sync.dma_start(bv_sb[:], bvrep.ap()[l])
        bo_sb = miscp.tile([1, D], BF16, name="bo_sb")
        nc.sync.dma_start(bo_sb[:], bo2.ap()[l])
        b1_sb = miscp.tile([128, FT], F32, name="b1_sb")
        nc.sync.dma_start(b1_sb[:], b1.ap()[l])
        b2_sb = miscp.tile([1, D], BF16, name="b2_sb")
        nc.sync.dma_start(b2_sb[:], b22.ap()[l])

        attn_scope = tc.tile_pool(name=f"l{l}_qkv", bufs=1)
        qkvp = attn_scope.__enter__()
        qsT = qkvp.tile([128, JT, S], BF16, name="qsT")
        kT = qkvp.tile([128, JT, S], BF16, name="kT")
        v_sb = qkvp.tile([128, TT, c.NHL * 65], BF16, name="v_sb")
        PW = 2 * c.SPAN + 256   # pos tables padded 128 each side (clamp ext)
        poskr = qkvp.tile([128, JT, PW], BF16, name="poskr")
        posq = qkvp.tile([128, JT, PW], BF16, name="posq")
        # ---- phase A: pos tables, qkv/v projections ----
        with (
            tc.tile_pool(name=f"l{l}_rel", bufs=1) as relp,
            tc.tile_pool(name=f"l{l}_wqk", bufs=1) as wqkp,
            tc.tile_pool(name=f"l{l}_wv", bufs=1) as wvpool,
            tc.tile_pool(name=f"l{l}_pps", bufs=2, space="PSUM") as pps,
            tc.tile_pool(name=f"l{l}_ppsb", bufs=1, space="PSUM") as ppsb,
        ):
            # load all q/k weight col-tiles once (shared by pos + qkv proj)
            qkwt = []
            for proj in range(2):
                row = []
                for jt in range(JT):
                    wof = proj * c.DCL + jt * 128
                    wt = wqkp.tile([128, DT, 128], BF16, tag=f"wqk{proj}{jt}",
                                   name=f"wqk{proj}{jt}")
                    nc.sync.dma_start(wt[:], wqkv.ap()[l, :, :, wof:wof + 128])
                    row.append(wt)
                qkwt.append(row)

            # pos projections: pos_kT_rev from relTr/Wk, pos_qT(scaled) from
            # relT/Wq.  rel chunks loaded once per (table, chunk).
            for which, (dst, reltab, proj, pofs, scl) in enumerate(
                (
                    (poskr, relTr, 1, JT, 1.0),        # Wk part, bias bk
                    (posq, relT, 0, 0, c.scale),       # Wq part, bias bq*s
                )
            ):
                rts = []
                for uc in range(c.NUC):
                    rt = relp.tile([128, DT, c.CU], BF16, tag=f"rel{uc}")
                    nc.sync.dma_start(
                        rt[:], reltab.ap()[:, :, uc * c.CU:(uc + 1) * c.CU])
                    rts.append(rt)
                for jt in range(JT):
                    pss = [pps.tile([128, c.CU], F32, tag=f"pos{uc}", name=f"pos{uc}")
                           for uc in range(c.NUC)]
                    mm_acc_multi(nc, [p[:] for p in pss], qkwt[proj][jt][:],
                                 [r[:] for r in rts], DT, True, True)
                    for uc in range(c.NUC):
                        nc.scalar.activation(
                            dst[:, jt, 128 + uc * c.CU:128 + (uc + 1) * c.CU],
                            pss[uc][:],
                            AF.Identity, bias=bq_sb[:, pofs + jt:pofs + jt + 1],
                            scale=scl)
                nc.vector.tensor_scalar_mul(
                    dst[:, :, 0:128],
                    dst[:, :, 128:129].to_broadcast((128, JT, 128)), 1.0)
                nc.vector.tensor_scalar_mul(
                    dst[:, :, PW - 128:PW],
                    dst[:, :, PW - 129:PW - 128].to_broadcast((128, JT, 128)),
                    1.0)

            # qkv projections (feature-major q/k; token-major v),
            # chunk-outer so chunk 0's projections start right after LN2's
            # chunk 0 instead of waiting for the whole LN.
            for ch in range(NCH):
                for proj in range(2):  # 0=q, 1=k
                    dst = (qsT, kT)[proj]
                    scl = (c.scale, 1.0)[proj]
                    for jt in range(JT):
                        ps = ppsb.tile([128, CH], F32, tag=f"qkv{ch}",
                                       name=f"qkv{ch}")
                        mm_acc(nc, ps[:], qkwt[proj][jt][:],
                               hTbf[:, :, ch * CH:(ch + 1) * CH], DT,
                               True, True)
                        nc.scalar.activation(
                            dst[:, jt, ch * CH:(ch + 1) * CH], ps[:],
                            AF.Identity,
                            bias=bq_sb[:, proj * JT + jt:proj * JT + jt + 1],
                            scale=scl)
            # v: out[token, dv_loc]
            wtv = wvpool.tile([128, DT, c.DCL], BF16, tag="wv")
            nc.sync.dma_start(wtv[:], wqkv.ap()[l, :, :, 2 * c.DCL:3 * c.DCL])
            for tt in range(TT):
                ps = ppsb.tile([128, c.DCL], F32, tag="vproj")
                mm_acc(nc, ps[:], hTbf[:, :, tt * 128:(tt + 1) * 128],
                       wtv[:], DT, True, True)
                for hl in range(c.NHL):
                    nc.vector.tensor_tensor(
                        v_sb[:, tt, hl * 65:hl * 65 + 64],
                        ps[:, hl * 64:(hl + 1) * 64],
                        bv_sb[:, hl * 64:(hl + 1) * 64], OP.add)
            for hl in range(c.NHL):
                nc.vector.memset(v_sb[:, :, hl * 65 + 64:hl * 65 + 65], 1.0)

        # ---- phase B: per-head attention ----
        with (
            tc.tile_pool(name=f"l{l}_ct", bufs=4) as ctp,
            tc.tile_pool(name=f"l{l}_g1", bufs=3) as g1p,
            tc.tile_pool(name=f"l{l}_g2", bufs=3) as g2p,
            tc.tile_pool(name=f"l{l}_ex", bufs=2) as exp_,
            tc.tile_pool(name=f"l{l}_sc", bufs=2) as scp,
            tc.tile_pool(name=f"l{l}_bps", bufs=1, space="PSUM") as bps,
            tc.tile_pool(name=f"l{l}_bsc", bufs=2, space="PSUM") as bsc,
            tc.tile_pool(name=f"l{l}_bp2", bufs=1, space="PSUM") as bps2,
            tc.tile_pool(name=f"l{l}_bp3", bufs=1, space="PSUM") as bps3,
        ):
            MW = 2 * c.SPAN + 256     # widened mid (covers +-128 clamp)
            ML = S - c.SPAN - 128      # mid left col in the table

            def build_tables(hl):
                """Emit table-build matmuls + staging + DRAM writes + clamp
                pads for head hl.  Returns the two DRAM table tiles."""
                jt, rb = hl // 2, 64 * (hl % 2)
                qh = qsT[rb:rb + 64, jt]      # [64, S]
                kh = kT[rb:rb + 64, jt]
                pkh = poskr[rb:rb + 64, jt]   # [64, 2*SPAN]
                pqh = posq[rb:rb + 64, jt]

                cq_dr = dramp.tile([S, c.W], FP8, tag="cq", name=f"cq{l}_{hl}")
                ck_dr = dramp.tile([S, c.W], FP8, tag="ck", name=f"ck{l}_{hl}")
                # c2p table: rows q, mid cols = q_s . pos_k_rev; p2c: rows k
                for which, (dr, lh, rh) in enumerate(
                        ((cq_dr, qh, pkh), (ck_dr, kh, pqh))):
                    th = dr[:].tensor
                    base = dr[:].offset
                    for rt in range(TT):
                        st = ctp.tile([128, MW], FP8, tag="cstage")
                        # one 1024-wide psum over the real (unclamped) rel
                        # range; the 128-col clamp flanks are broadcast on
                        # the DVE afterwards.
                        ps = bps.tile([128, 1024], F32, tag=f"ctab{which}")
                        for co in range(2):
                            nc.tensor.matmul(
                                ps[:, co * 512:(co + 1) * 512],
                                lhsT=lh[:, rt * 128:(rt + 1) * 128],
                                rhs=rh[:, 128 + co * 512:128 + (co + 1) * 512],
                                start=True, stop=True)
                        if which == 0:
                            nc.scalar.activation(
                                st[:, 128:1152], ps[:], AF.Copy, scale=256.0)
                        else:
                            nc.vector.tensor_scalar_mul(
                                st[:, 128:1152], ps[:], 256.0)
                        nc.vector.tensor_scalar_mul(
                            st[:, 0:128],
                            st[:, 128:129].to_broadcast((128, 128)), 1.0)
                        nc.vector.tensor_scalar_mul(
                            st[:, 1152:1280],
                            st[:, 1151:1152].to_broadcast((128, 128)), 1.0)
                        dst = bass.AP(
                            th, base + (rt * 128) * c.W + ML,
                            [[c.W, 128], [1, MW]])
                        nc.sync.dma_start(dst, st[:])
                    # log-doubling clamp pads (row-constant regions)
                    pos, havew = ML, 128
                    while pos > 0:
                        w = min(pos, havew)
                        ldst = bass.AP(th, base + pos - w, [[c.W, S], [1, w]])
                        lsrc = bass.AP(th, base + pos, [[c.W, S], [1, w]])
                        nc.sync.dma_start(ldst, lsrc)
                        pos -= w
                        havew += w
                    pos, havew = ML + MW, 128
                    while pos < c.W:
                        w = min(c.W - pos, havew)
                        rdst = bass.AP(th, base + pos, [[c.W, S], [1, w]])
                        rsrc = bass.AP(th, base + pos - havew,
                                       [[c.W, S], [1, w]])
                        nc.sync.dma_start(rdst, rsrc)
                        pos += w
                        havew += w
                return cq_dr, ck_dr

            def attend(hl, cq_dr, ck_dr):
                """Scores + softmax + probs@v for head hl from its tables."""
                jt, rb = hl // 2, 64 * (hl % 2)
                qh = qsT[rb:rb + 64, jt]      # [64, S]
                kh = kT[rb:rb + 64, jt]

                g1 = g1p.tile([128, TT, S], FP8, tag="g1")
                thq = cq_dr[:].tensor
                bq_ = cq_dr[:].offset
                for qt in range(TT):
                    src = bass.AP(thq, bq_ + (c.W - 1) * (qt * 128) + S - 1,
                                  [[c.W - 1, 128], [1, S]])
                    nc.sync.dma_start(g1[:, qt], src)

                ex = exp_.tile([128, TT, S], BF16, tag="ex")
                thk = ck_dr[:].tensor
                bk_ = ck_dr[:].offset
                for kt in range(TT):
                    g2 = g2p.tile([128, S], FP8, tag="g2", name=f"g2_{kt}")
                    src = bass.AP(thk, bk_ + (c.W - 1) * (kt * 128) + S,
                                  [[c.W - 1, 128], [1, S]])
                    nc.sync.dma_start(g2[:], src)
                    for ch in range(NCH):
                        ps = bsc.tile([128, CH], F32, tag="scores")
                        nc.tensor.matmul(
                            ps[:], lhsT=kh[:, kt * 128:(kt + 1) * 128],
                            rhs=qh[:, ch * CH:(ch + 1) * CH],
                            start=True, stop=False)
                        nc.tensor.matmul(
                            ps[:], lhsT=ident8[:],
                            rhs=g2[:, ch * CH:(ch + 1) * CH],
                            start=False, stop=False)
                        nq = CH // 128
                        for qi in range(nq):
                            qt = ch * nq + qi
                            nc.tensor.matmul(
                                ps[:, qi * 128:(qi + 1) * 128],
                                lhsT=g1[:, qt, kt * 128:(kt + 1) * 128],
                                rhs=ident8[:],
                                start=False, stop=True,
                                skip_group_check=(qi != nq - 1))
                        nc.scalar.activation(
                            ex[:, kt, ch * CH:(ch + 1) * CH], ps[:], AF.Exp,
                            bias=mb_sb[:, kt:kt + 1], scale=1.0)

                for ch in range(NCH):
                    pv = bps2.tile([65, CH], F32, tag="pv")
                    for kt in range(TT):
                        nc.tensor.matmul(
                            pv[:], lhsT=v_sb[:, kt, hl * 65:hl * 65 + 65],
                            rhs=ex[:, kt, ch * CH:(ch + 1) * CH],
                            start=(kt == 0), stop=(kt == TT - 1))
                    rec = scp.tile([1, CH], BF16, tag="rec")
                    with nc.allow_low_precision(reason="softmax denom bf16"):
                        nc.vector.reciprocal(rec[:], pv[64:65, :])
                    pb = bps3.tile([64, CH], F32, tag="recb")
                    nc.tensor.matmul(pb[:], lhsT=ones1x64[:], rhs=rec[:],
                                     start=True, stop=True)
                    rb_sb = scp.tile([64, CH], F32, tag="recbs")
                    nc.scalar.copy(rb_sb[:], pb[:])
                    nc.vector.tensor_tensor(
                        ctxT[rb:rb + 64, jt, ch * CH:(ch + 1) * CH],
                        pv[0:64, :], rb_sb[:], OP.mult)

            # software-pipeline heads: the PE queue is in-order, so head
            # h's score matmuls must not sit at the queue head while h's
            # table DRAM roundtrip is still in flight — keep LOOKAHEAD
            # heads of table builds queued ahead.
            LOOKAHEAD = 2
            built = {}
            for j in range(min(LOOKAHEAD + 1, c.NHL)):
                built[j] = build_tables(j)
            for hl in range(c.NHL):
                attend(hl, *built.pop(hl))
                nxt = hl + LOOKAHEAD + 1
                if nxt < c.NHL:
                    built[nxt] = build_tables(nxt)

        attn_scope.__exit__(None, None, None)   # free qsT/kT/v/pos SBUF

        # ---- phase C: Wo + AR + LN1 ----
        # per-token-chunk AR tensors: chunk 0's reduce+LN runs while chunk 1
        # is still accumulating
        ar1 = [dramp.tile([128, DT, CH], BF16, tag=f"arin{ch}",
                          name=f"ar1i_{l}_{ch}") for ch in range(NCH)]
        ar1o = [dramp.tile([128, DT, CH], BF16, tag=f"arout{ch}",
                           name=f"ar1o_{l}_{ch}") for ch in range(NCH)]
        with (
            tc.tile_pool(name=f"l{l}_wops", bufs=1, space="PSUM") as wops,
            tc.tile_pool(name=f"l{l}_wost", bufs=3) as wost,
            tc.tile_pool(name=f"l{l}_wo", bufs=1) as wopool,
        ):
            wos = wopool.tile([128, JT, D], BF16, tag="wo")
            nc.sync.dma_start(wos[:], wo.ap()[l])
            # chunk-outer so chunk 0's AllReduce fires while chunk 1 runs
            for ch in range(NCH):
                for dt in range(DT):
                    ps = wops.tile([128, CH], F32, tag=f"wo{ch}",
                                   name=f"wo{ch}")
                    mm_acc(nc, ps[:], wos[:, :, dt * 128:(dt + 1) * 128],
                           ctxT[:, :, ch * CH:(ch + 1) * CH], JT, True, False)
                    nc.tensor.matmul(
                        ps[:], lhsT=bo_sb[:, dt * 128:(dt + 1) * 128],
                        rhs=onesrow[:], start=False, stop=True)
                    st = wost.tile([128, CH], BF16, tag="wost")
                    nc.vector.scalar_tensor_tensor(
                        st[:], hTbf[:, dt, ch * CH:(ch + 1) * CH], 0.5,
                        ps[:], OP.mult, OP.add)
                    nc.sync.dma_start(ar1[ch][:, dt], st[:])
                if c.n_cores == 1 or c.no_cc:
                    nc.sync.dma_start(ar1o[ch][:], ar1[ch][:])
                else:
                    nc.gpsimd.collective_compute(
                        "AllReduce", OP.add, replica_groups=pairs,
                        ins=[ar1[ch].opt()], outs=[ar1o[ch].opt()])
        _ln(nc, tc, c, l, ar1o, hTbf, ln1g, ln1b, onesb, eps2,
            invD_row, D_row, onesf)

        # ---- phase D: FFN + AR + LN2 ----
        ar2 = [dramp.tile([128, DT, CH], BF16, tag=f"arin{ch}",
                          name=f"ar2i_{l}_{ch}") for ch in range(NCH)]
        ar2o = [dramp.tile([128, DT, CH], BF16, tag=f"arout{ch}",
                           name=f"ar2o_{l}_{ch}") for ch in range(NCH)]
        with (
            tc.tile_pool(name=f"l{l}_gt", bufs=2) as gtp,
            tc.tile_pool(name=f"l{l}_w1", bufs=1) as w1pool,
            tc.tile_pool(name=f"l{l}_w2", bufs=1) as w2pool,
            tc.tile_pool(name=f"l{l}_f1ps", bufs=1, space="PSUM") as f1ps,
            tc.tile_pool(name=f"l{l}_f2ps", bufs=1, space="PSUM") as f2ps,
            tc.tile_pool(name=f"l{l}_fst", bufs=3) as fst,
        ):
            w1t = w1pool.tile([128, DT, c.FL], BF16, tag="w1")
            nc.sync.dma_start(w1t[:], w1.ap()[l])
            w2t = w2pool.tile([128, FT, D], BF16, tag="w2")
            nc.sync.dma_start(w2t[:], w2.ap()[l])
            gts = [gtp.tile([128, FT, CH], BF16, tag="gt", name=f"gt{ch}")
                   for ch in range(NCH)]
            # chunk-outer: chunk 0 flows W1 -> W2 -> AR while chunk 1
            # computes, so the AllReduce latency overlaps compute.
            for ch in range(NCH):
                hchunk = hTbf[:, :, ch * CH:(ch + 1) * CH]
                for ft in range(FT):
                    ps = f1ps.tile([128, CH], F32, tag=f"f1{ch}",
                                   name=f"f1{ch}")
                    mm_acc(nc, ps[:], w1t[:, :, ft * 128:(ft + 1) * 128],
                           hchunk, DT, True, True)
                    nc.scalar.activation(
                        gts[ch][:, ft], ps[:],
                        AF.Gelu if c.act == "gelu" else AF.Relu,
                        bias=b1_sb[:, ft:ft + 1], scale=1.0)
                for dt in range(DT):
                    ps = f2ps.tile([128, CH], F32, tag=f"f2{ch}",
                                   name=f"f2{ch}")
                    mm_acc(nc, ps[:], w2t[:, :, dt * 128:(dt + 1) * 128],
                           gts[ch][:], FT, True, False)
                    nc.tensor.matmul(
                        ps[:], lhsT=b2_sb[:, dt * 128:(dt + 1) * 128],
                        rhs=onesrow[:], start=False, stop=True)
                    st = fst.tile([128, CH], BF16, tag="fst")
                    nc.vector.scalar_tensor_tensor(
                        st[:], hTbf[:, dt, ch * CH:(ch + 1) * CH], 0.5,
                        ps[:], OP.mult, OP.add)
                    nc.sync.dma_start(ar2[ch][:, dt], st[:])
                if c.n_cores == 1 or c.no_cc:
                    nc.sync.dma_start(ar2o[ch][:], ar2[ch][:])
                else:
                    nc.gpsimd.collective_compute(
                        "AllReduce", OP.add, replica_groups=pairs,
                        ins=[ar2[ch].opt()], outs=[ar2o[ch].opt()])
        _ln(nc, tc, c, l, ar2o, hTbf, ln2g, ln2b, onesb, eps2,
            invD_row, D_row, onesf)


def _ln(nc, tc, c, l, x_drs, hTbf, g_in, b_in, onesb, eps2,
        invD_row, D_row, onesf):
    """Feature-major layernorm over partitions: x in per-chunk DRAM tiles
    [128, DT, CH] bf16 -> hTbf.  Single pass over x: tiles are kept in SBUF
    between the stats accumulation and the normalize step.
    rstd computed as D/sqrt(D*s1 - s0^2 + D^2*eps)."""
    S, CH, NCH, DT, SUB = c.S, c.CH, c.NCH, c.DT, c.SUB
    with (
        tc.tile_pool(name=f"ln{l}", bufs=2) as lp,
        tc.tile_pool(name=f"ln{l}s", bufs=1) as lps,
        tc.tile_pool(name=f"ln{l}ps", bufs=1, space="PSUM") as pps,
        tc.tile_pool(name=f"ln{l}pb", bufs=2, space="PSUM") as pbs,
    ):
        g_sb = lps.tile([128, DT], F32, tag="g")
        nc.sync.dma_start(g_sb[:], g_in.ap()[l])
        b_sb = lps.tile([128, DT], F32, tag="b")
        nc.sync.dma_start(b_sb[:], b_in.ap()[l])

        stats0 = pps.tile([1, S], F32, tag="stats0")
        stats1 = pps.tile([1, S], F32, tag="stats1")
        ngr = DT // SUB
        s0 = lps.tile([1, S], F32, tag="s0")
        s0b = lps.tile([1, S], BF16, tag="s0b")
        s1 = lps.tile([1, S], F32, tag="s1")
        u = lps.tile([1, S], F32, tag="u")
        rp = lps.tile([1, S], BF16, tag="rp")
        mu_b = lps.tile([128, S], F32, tag="mub")
        rs_b = lps.tile([128, S], F32, tag="rsb")
        # fully per-chunk: stats(ch) -> rstd(ch) -> bcast(ch) ->
        # normalize(ch) before chunk ch+1's stats, so chunk 0's output
        # unblocks downstream consumers while chunk 1 still reduces.
        for ch in range(NCH):
            xts = {}
            for g in range(ngr):
                xt = lps.tile([128, SUB, CH], BF16, tag=f"x{ch}_{g}")
                xts[g] = xt
                nc.sync.dma_start(
                    xt[:], x_drs[ch][:, g * SUB:(g + 1) * SUB])
                x2 = lp.tile([128, SUB, CH], BF16, tag="x2")
                nc.scalar.square(x2[:], xt[:])
                for s in range(SUB):
                    nc.tensor.matmul(
                        stats0[:, ch * CH:(ch + 1) * CH], lhsT=onesb[:, s],
                        rhs=xt[:, s], start=(g == 0 and s == 0),
                        stop=(g == ngr - 1 and s == SUB - 1))
                    nc.tensor.matmul(
                        stats1[:, ch * CH:(ch + 1) * CH], lhsT=onesb[:, s],
                        rhs=x2[:, s], start=(g == 0 and s == 0),
                        stop=(g == ngr - 1 and s == SUB - 1))
            cs = slice(ch * CH, (ch + 1) * CH)
            nc.scalar.copy(s0[:, cs], stats0[:, cs])
            nc.scalar.copy(s1[:, cs], stats1[:, cs])
            nc.vector.tensor_copy(s0b[:, cs], s0[:, cs])
            nc.vector.tensor_tensor(u[:, cs], s0[:, cs], s0[:, cs], OP.mult)
            nc.vector.scalar_tensor_tensor(
                u[:, cs], s1[:, cs], float(c.D), u[:, cs], OP.mult, OP.subtract)
            nc.scalar.activation(u[:, cs], u[:, cs], AF.Sqrt, bias=eps2[:],
                                 scale=1.0)
            with nc.allow_low_precision(reason="rstd bf16 broadcast"):
                nc.vector.reciprocal(rp[:, cs], u[:, cs])
            pm = pbs.tile([128, CH], F32, tag="bc")
            nc.tensor.matmul(pm[:], lhsT=invD_row[:], rhs=s0b[0:1, cs],
                             start=True, stop=True)
            nc.scalar.copy(mu_b[:, cs], pm[:])
            pr = pbs.tile([128, CH], F32, tag="bc")
            nc.tensor.matmul(pr[:], lhsT=D_row[:], rhs=rp[0:1, cs],
                             start=True, stop=True)
            nc.scalar.copy(rs_b[:, cs], pr[:])

            for g in range(ngr):
                xt = xts[g]
                xn = lp.tile([128, SUB, CH], F32, tag="xn")
                mub = mu_b[:, None, ch * CH:(ch + 1) * CH].to_broadcast(
                    (128, SUB, CH))
                nc.vector.tensor_tensor(xn[:], xt[:], mub, OP.subtract)
                rsb = rs_b[:, None, ch * CH:(ch + 1) * CH].to_broadcast(
                    (128, SUB, CH))
                nc.vector.tensor_tensor(xn[:], xn[:], rsb, OP.mult)
                for i in range(SUB):
                    dt = g * SUB + i
                    nc.scalar.activation(
                        hTbf[:, dt, ch * CH:(ch + 1) * CH], xn[:, i],
                        AF.Identity, bias=b_sb[:, dt:dt + 1],
                        scale=g_sb[:, dt:dt + 1])


# ---------------------------------------------------------------------------
# host side
# ---------------------------------------------------------------------------

def host_prep(c, inputs):
    """Build per-core in_maps from full inputs."""
    bf = ml_dtypes.bfloat16
    f32 = np.float32
    ii = {k: np.asarray(v) for k, v in inputs.items()}
    S, D, L = c.S, c.D, c.L

    def tokmaj(vec):  # [S] -> [128, TT]   t = tt*128 + p
        return np.ascontiguousarray(vec.reshape(c.TT, 128).T)

    def dpart(vec):  # [D] -> [128, DT]
        return np.ascontiguousarray(vec.reshape(c.DT, 128).T)

    rel = ii["rel_emb"].astype(f32)  # [2*SPAN, D]
    relT = np.ascontiguousarray(
        rel.T.reshape(c.DT, 128, 2 * c.SPAN).transpose(1, 0, 2)).astype(bf)
    relr = rel[::-1]
    relTr = np.ascontiguousarray(
        relr.T.reshape(c.DT, 128, 2 * c.SPAN).transpose(1, 0, 2)).astype(bf)

    in_maps = []
    for core in range(c.n_cores):
        b, half = core // 2, core % 2
        colr = slice(half * c.DCL, (half + 1) * c.DCL)
        fcol = slice(half * c.FL, (half + 1) * c.FL)

        ids = ii["input_ids"][b].astype(np.int64)
        w = np.zeros((16, S // 16), np.int16)
        for i in range(S):
            w[i % 16, i // 16] = ids[i]
        ids16 = np.tile(w, (8, 1))

        seg = ii["segment_ids"][b].astype(f32)
        mask = ii["attention_mask"][b].astype(f32)

        wq = ii["Wq"][:, :, colr].astype(f32)
        wk = ii["Wk"][:, :, colr].astype(f32)
        wv = ii["Wv"][:, :, colr].astype(f32)
        wqkv = np.concatenate([wq, wk, wv], axis=2)  # [L, D, 3*DCL]
        wqkv = wqkv.reshape(L, c.DT, 128, 3 * c.DCL).transpose(0, 2, 1, 3)

        bq = ii["bq"][:, colr].astype(f32) * c.scale
        bk = ii["bk"][:, colr].astype(f32)
        bv = ii["bv"][:, colr].astype(f32)
        bqkv = np.concatenate(
            [bq.reshape(L, c.JT, 128).transpose(0, 2, 1),
             bk.reshape(L, c.JT, 128).transpose(0, 2, 1),
             np.zeros((L, 128, c.JT), f32)], axis=2)
        bvrep = np.broadcast_to(bv[:, None, :], (L, 128, c.DCL))

        wo_ = ii["Wo"][:, colr, :].astype(f32)
        wo_ = wo_.reshape(L, c.JT, 128, D).transpose(0, 2, 1, 3)
        bo2 = (ii["bo"].astype(f32) / 2.0)[:, None, :]

        w1_ = ii["W1"][:, :, fcol].astype(f32)
        w1_ = w1_.reshape(L, c.DT, 128, c.FL).transpose(0, 2, 1, 3)
        b1_ = ii["b1"][:, fcol].astype(f32).reshape(L, c.FT, 128).transpose(0, 2, 1)
        w2_ = ii["W2"][:, fcol, :].astype(f32)
        w2_ = w2_.reshape(L, c.FT, 128, D).transpose(0, 2, 1, 3)
        b22 = (ii["b2"].astype(f32) / 2.0)[:, None, :]

        m = {
            "ids16": ids16,
            "tok_emb": ii["tok_emb"].astype(f32),
            "segsel": tokmaj(seg),
            "seg0rep": np.broadcast_to(
                ii["seg_emb"][0].astype(f32), (128, D)).copy(),
            "segdrep": np.broadcast_to(
                (ii["seg_emb"][1] - ii["seg_emb"][0]).astype(f32),
                (128, D)).copy(),
            "maskt": tokmaj(mask),
            "maskbias": tokmaj(NEG * (1.0 - mask)),
            "egrep": np.broadcast_to(
                ii["emb_ln_g"].astype(f32), (128, D)).copy(),
            "ebrep": np.broadcast_to(
                ii["emb_ln_b"].astype(f32), (128, D)).copy(),
            "relT": relT,
            "relTr": relTr,
            "wqkv": wqkv.astype(bf),
            "bqkv": np.ascontiguousarray(bqkv),
            "bvrep": np.ascontiguousarray(bvrep),
            "wo": wo_.astype(bf),
            "bo2": bo2.astype(bf),
            "w1": w1_.astype(bf),
            "b1": np.ascontiguousarray(b1_),
            "w2": w2_.astype(bf),
            "b22": b22.astype(bf),
            "ln1g": ii["ln1_g"].astype(f32).reshape(
                L, c.DT, 128).transpose(0, 2, 1),
            "ln1b": ii["ln1_b"].astype(f32).reshape(
                L, c.DT, 128).transpose(0, 2, 1),
            "ln2g": ii["ln2_g"].astype(f32).reshape(
                L, c.DT, 128).transpose(0, 2, 1),
            "ln2b": ii["ln2_b"].astype(f32).reshape(
                L, c.DT, 128).transpose(0, 2, 1),
        }
        m = {k: np.ascontiguousarray(v) for k, v in m.items()}
        in_maps.append(m)
    return in_maps


def assemble(c, results):
    """results[core]["out_hT"] [128, DT, S] -> [B, S, D] fp32."""
    out = np.zeros((c.B, c.S, c.D), np.float32)
    for b in range(c.B):
        hT = results[2 * b]["out_hT"]  # [128, DT, S]
        out[b] = hT.transpose(2, 1, 0).reshape(c.S, c.D)
    return out


_nc_cache = {}


def _get_nc(c):
    key = (c.B, c.S, c.D, c.H, c.F, c.L, c.V, c.SPAN, c.n_cores)
    if key not in _nc_cache:
        _nc_cache[key] = build_nc(c)
    return _nc_cache[key]


def kernel(**inputs):
    from concourse import bass_utils
    c = Cfg()
    nc = _get_nc(c)
    in_maps = host_prep(c, inputs)
    res = bass_utils.run_bass_kernel_spmd(
        nc, in_maps, core_ids=list(range(c.n_cores)))
    return assemble(c, res.results)



# revision 62
# speedup vs baseline: 1.1329x; 1.0170x over previous
"""DeBERTa-bare Trainium2 Bass kernel.

Topology: 8 NeuronCores = 4 data-parallel pairs (one batch element each) x
2-way tensor parallel (heads + FFN split) with pairwise AllReduce.

Everything on-chip runs feature-major ("transposed"): h is kept as
hT[d, token].  The DeBERTa disentangled-attention gathers
(take_along_axis over relative positions) are realized as affine "skew"
access-pattern DMA reads from DRAM-resident, clamp-extended c2p/p2c tables
(fp8, x256 scaled), injected into the score PSUM via scaled-identity
matmuls.
"""

import sys

for _p in ("/opt/trn_rl_repo",):
    if _p not in sys.path:
        sys.path.insert(0, _p)

import numpy as np
import ml_dtypes

import concourse.bass as bass
import concourse.bacc as bacc
import concourse.tile as tile
import concourse.mybir as mybir
from concourse.masks import make_identity

F32 = mybir.dt.float32
BF16 = mybir.dt.bfloat16
FP8 = mybir.dt.float8e4
I16 = mybir.dt.int16

AF = mybir.ActivationFunctionType
OP = mybir.AluOpType

NEG = -1e9


def mm_acc(nc, ps, lhsT3, rhs3, nsub, start, stop):
    """Accumulating matmul over `nsub` 128-contraction subtiles.
    lhsT3/rhs3: APs shaped [128, nsub, *]."""
    for s in range(nsub):
        nc.tensor.matmul(ps, lhsT3[:, s], rhs3[:, s],
                         start=(start and s == 0), stop=(stop and s == nsub - 1))


def mm_acc_multi(nc, pss, lhsT3, rhss, nsub, start, stop):
    """Like mm_acc but for several moving operands sharing the stationary
    subtiles: subtile-outer order so each lhsT subtile is loaded once."""
    for s in range(nsub):
        for i, (ps, rhs3) in enumerate(zip(pss, rhss)):
            nc.tensor.matmul(ps, lhsT3[:, s], rhs3[:, s],
                             start=(start and s == 0),
                             stop=(stop and s == nsub - 1))


class Cfg:
    def __init__(self, B=4, S=1024, D=1024, H=16, F=4096, L=4, V=32000, SPAN=512,
                 n_cores=8, act="gelu", no_cc=False):
        self.B, self.S, self.D, self.H, self.F, self.L, self.V, self.SPAN = (
            B, S, D, H, F, L, V, SPAN)
        self.n_cores = n_cores
        self.DH = D // H
        assert self.DH == 64
        self.DT = D // 128          # d tiles
        self.TT = S // 128          # token tiles
        self.NHL = H // 2           # heads per core
        self.DCL = self.NHL * self.DH   # local head-dim cols
        self.JT = self.DCL // 128   # local dcol tiles (2 heads per tile)
        self.FL = F // 2            # local ffn cols
        self.FT = self.FL // 128
        self.CH = min(512, S)       # token chunk
        self.NCH = S // self.CH
        self.CU = min(512, 2 * SPAN)
        self.NUC = (2 * SPAN) // self.CU
        self.SUB = min(4, self.DT)
        self.FSUB = min(4, self.FT)
        self.W = 2 * S              # extended table width
        self.scale = 1.0 / np.sqrt(3.0 * self.DH)
        self.act = act
        self.no_cc = no_cc


def build_nc(cfg):
    c = cfg
    nc = bacc.Bacc("TRN2", target_bir_lowering=False, debug=False,
                   num_devices=c.n_cores)

    def inp(name, shape, dt):
        return nc.dram_tensor(name, list(shape), dt, kind="ExternalInput")

    ids16 = inp("ids16", [128, c.S // 16], I16)
    tok_emb = inp("tok_emb", [c.V, c.D], F32)
    segsel = inp("segsel", [128, c.TT], F32)
    seg0rep = inp("seg0rep", [128, c.D], F32)
    segdrep = inp("segdrep", [128, c.D], F32)
    maskt = inp("maskt", [128, c.TT], F32)
    maskbias = inp("maskbias", [128, c.TT], F32)
    egrep = inp("egrep", [128, c.D], F32)
    ebrep = inp("ebrep", [128, c.D], F32)
    relT = inp("relT", [128, c.DT, 2 * c.SPAN], BF16)
    relTr = inp("relTr", [128, c.DT, 2 * c.SPAN], BF16)
    wqkv = inp("wqkv", [c.L, 128, c.DT, 3 * c.DCL], BF16)
    bqkv = inp("bqkv", [c.L, 128, 3 * c.JT], F32)
    bvrep = inp("bvrep", [c.L, 128, c.DCL], F32)
    wo = inp("wo", [c.L, 128, c.JT, c.D], BF16)
    bo2 = inp("bo2", [c.L, 1, c.D], BF16)
    w1 = inp("w1", [c.L, 128, c.DT, c.FL], BF16)
    b1 = inp("b1", [c.L, 128, c.FT], F32)
    w2 = inp("w2", [c.L, 128, c.FT, c.D], BF16)
    b22 = inp("b22", [c.L, 1, c.D], BF16)
    ln1g = inp("ln1g", [c.L, 128, c.DT], F32)
    ln1b = inp("ln1b", [c.L, 128, c.DT], F32)
    ln2g = inp("ln2g", [c.L, 128, c.DT], F32)
    ln2b = inp("ln2b", [c.L, 128, c.DT], F32)

    out_hT = nc.dram_tensor("out_hT", [128, c.DT, c.S], F32, kind="ExternalOutput")

    pairs = [[2 * i, 2 * i + 1] for i in range(c.n_cores // 2)]

    with tile.TileContext(nc) as tc:
        import contextlib
        est = contextlib.ExitStack()
        with est:
            const = est.enter_context(tc.tile_pool(name="const", bufs=1))
            resid = est.enter_context(tc.tile_pool(name="resid", bufs=1))
            dramp = est.enter_context(tc.tile_pool(name="dramp", bufs=3, space="DRAM"))
            wpool = est.enter_context(tc.tile_pool(name="wpool", bufs=4))

            identT = const.tile([128, 128], F32)
            make_identity(nc, identT[:])
            ident8 = const.tile([128, 128], FP8)
            nc.gpsimd.memset(ident8[:], 2.0 ** -8)
            nc.gpsimd.affine_select(
                out=ident8[:], in_=ident8[:], compare_op=OP.is_equal, fill=0.0,
                base=0, pattern=[[-1, 128]], channel_multiplier=1)
            ones1x64 = const.tile([1, 64], BF16)
            nc.vector.memset(ones1x64[:], 1.0)
            ones1x128 = const.tile([1, 128], F32)
            nc.vector.memset(ones1x128[:], 1.0)
            onesb = const.tile([128, c.SUB, 1], BF16)
            nc.vector.memset(onesb[:], 1.0)
            onesf = const.tile([128, c.SUB, 1], F32)
            nc.vector.memset(onesf[:], 1.0)
            onesrow = const.tile([1, c.CH], BF16)
            nc.vector.memset(onesrow[:], 1.0)
            eps1 = const.tile([1, 1], F32)
            nc.vector.memset(eps1[:], 1e-12)
            eps2 = const.tile([1, 1], F32)
            nc.vector.memset(eps2[:], float(c.D) ** 2 * 1e-12)
            invD_row = const.tile([1, 128], BF16)
            nc.vector.memset(invD_row[:], 1.0 / c.D)
            D_row = const.tile([1, 128], BF16)
            nc.vector.memset(D_row[:], float(c.D))
            mb_sb = const.tile([128, c.TT], F32)
            nc.sync.dma_start(mb_sb[:], maskbias.ap())

            hTbf = resid.tile([128, c.DT, c.S], BF16)

            # ---------------- embedding ----------------
            with (
                tc.tile_pool(name="embp", bufs=1) as embp,
                tc.tile_pool(name="embps", bufs=2, space="PSUM") as embps,
            ):
                ids_sb = embp.tile([128, c.S // 16], I16)
                nc.sync.dma_start(ids_sb[:], ids16.ap())
                gb = embp.tile([128, c.TT, c.D], F32)
                nc.gpsimd.dma_gather(
                    gb[:], tok_emb.ap(), ids_sb[:], num_idxs=c.S,
                    num_idxs_reg=c.S, elem_size=c.D)

                s0 = embp.tile([128, c.D], F32)
                nc.sync.dma_start(s0[:], seg0rep.ap())
                sd = embp.tile([128, c.D], F32)
                nc.sync.dma_start(sd[:], segdrep.ap())
                ssel = embp.tile([128, c.TT], F32)
                nc.sync.dma_start(ssel[:], segsel.ap())
                mt = embp.tile([128, c.TT], F32)
                nc.sync.dma_start(mt[:], maskt.ap())
                eg = embp.tile([128, c.D], F32)
                nc.sync.dma_start(eg[:], egrep.ap())
                eb = embp.tile([128, c.D], F32)
                nc.sync.dma_start(eb[:], ebrep.ap())

                # per-token-tile pipeline: token tile tt's transposes start
                # as soon as its stats are done instead of after the whole
                # batch of LN work.
                for tt in range(c.TT):
                    g1t = gb[:, tt]
                    nc.vector.tensor_tensor(
                        g1t, g1t, s0[:, None, :].to_broadcast((128, 1, c.D)),
                        OP.add)
                    nc.vector.scalar_tensor_tensor(
                        g1t, sd[:, None, :], ssel[:, tt:tt + 1], g1t,
                        OP.mult, OP.add)
                    mean = embp.tile([128, 1, 1], F32, tag=f"mean{tt}")
                    nc.vector.tensor_reduce(
                        mean[:], g1t, mybir.AxisListType.X, OP.add)
                    nc.vector.tensor_scalar_mul(mean[:], mean[:], 1.0 / c.D)
                    nc.vector.tensor_tensor(
                        g1t, g1t, mean[:].to_broadcast((128, 1, c.D)),
                        OP.subtract)
                    sq = embp.tile([128, 1, c.D], F32, tag=f"sq{tt}")
                    nc.scalar.square(sq[:], g1t)
                    var = embp.tile([128, 1, 1], F32, tag=f"var{tt}")
                    nc.vector.tensor_reduce(
                        var[:], sq[:], mybir.AxisListType.X, OP.add)
                    nc.vector.tensor_scalar(
                        var[:], var[:], 1.0 / c.D, 1e-12, OP.mult, OP.add)
                    rstd = embp.tile([128, 1, 1], F32, tag=f"rstd{tt}")
                    nc.vector.reciprocal(rstd[:], var[:])
                    nc.scalar.sqrt(rstd[:], rstd[:])
                    nc.vector.tensor_tensor(
                        g1t, g1t, rstd[:].to_broadcast((128, 1, c.D)), OP.mult)
                    nc.vector.tensor_tensor(
                        g1t, g1t, eg[:, None, :].to_broadcast((128, 1, c.D)),
                        OP.mult)
                    nc.vector.tensor_tensor(
                        g1t, g1t, eb[:, None, :].to_broadcast((128, 1, c.D)),
                        OP.add)
                    nc.vector.tensor_scalar_mul(g1t, g1t, mt[:, tt:tt + 1])

                    for dt in range(c.DT):
                        pst = embps.tile([128, 128], F32, tag="tp")
                        nc.tensor.transpose(
                            pst[:], gb[:, tt, dt * 128:(dt + 1) * 128], identT[:])
                        nc.vector.tensor_copy(
                            hTbf[:, dt, tt * 128:(tt + 1) * 128], pst[:])

            # ---------------- layers ----------------
            for l in range(c.L):
                layer(nc, tc, c, l, hTbf, mb_sb, dramp, wpool,
                      identT, ident8, ones1x64, ones1x128, onesb, eps1, onesrow,
                      eps2, invD_row, D_row, onesf,
                      wqkv, bqkv, bvrep, wo, bo2, w1, b1, w2, b22,
                      ln1g, ln1b, ln2g, ln2b, relT, relTr, pairs)

            with tc.tile_pool(name="outp", bufs=2) as outp:
                for dt in range(c.DT):
                    ot = outp.tile([128, c.S], F32, tag="o")
                    nc.scalar.copy(ot[:], hTbf[:, dt])
                    nc.sync.dma_start(out_hT.ap()[:, dt], ot[:])

    nc.compile()
    return nc


def layer(nc, tc, c, l, hTbf, mb_sb, dramp, wpool,
          identT, ident8, ones1x64, ones1x128, onesb, eps1, onesrow,
          eps2, invD_row, D_row, onesf,
          wqkv, bqkv, bvrep, wo, bo2, w1, b1, w2, b22,
          ln1g, ln1b, ln2g, ln2b, relT, relTr, pairs):
    S, D, CH, NCH = c.S, c.D, c.CH, c.NCH
    DT, TT, JT, FT, SUB = c.DT, c.TT, c.JT, c.FT, c.SUB

    with (
        tc.tile_pool(name=f"l{l}_ctx", bufs=1) as ctxp,
        tc.tile_pool(name=f"l{l}_misc", bufs=1) as miscp,
    ):
        ctxT = ctxp.tile([128, JT, S], BF16, name="ctxT")
        bq_sb = miscp.tile([128, 3 * JT], F32, name="bq_sb")
        nc.sync.dma_start(bq_sb[:], bqkv.ap()[l])
        bv_sb = miscp.tile([128, c.DCL], F32, name="bv_sb")
        nc.sync.dma_start(bv_sb[:], bvrep.ap()[l])
        bo_sb = miscp.tile([1, D], BF16, name="bo_sb")
        nc.sync.dma_start(bo_sb[:], bo2.ap()[l])
        b1_sb = miscp.tile([128, FT], F32, name="b1_sb")
        nc.sync.dma_start(b1_sb[:], b1.ap()[l])
        b2_sb = miscp.tile([1, D], BF16, name="b2_sb")
        nc.sync.dma_start(b2_sb[:], b22.ap()[l])

        attn_scope = tc.tile_pool(name=f"l{l}_qkv", bufs=1)
        qkvp = attn_scope.__enter__()
        qsT = qkvp.tile([128, JT, S], BF16, name="qsT")
        kT = qkvp.tile([128, JT, S], BF16, name="kT")
        v_sb = qkvp.tile([128, TT, c.NHL * 65], BF16, name="v_sb")
        PW = 2 * c.SPAN + 256   # pos tables padded 128 each side (clamp ext)
        poskr = qkvp.tile([128, JT, PW], BF16, name="poskr")
        posq = qkvp.tile([128, JT, PW], BF16, name="posq")
        # ---- phase A: pos tables, qkv/v projections ----
        with (
            tc.tile_pool(name=f"l{l}_rel", bufs=1) as relp,
            tc.tile_pool(name=f"l{l}_wqk", bufs=1) as wqkp,
            tc.tile_pool(name=f"l{l}_wv", bufs=1) as wvpool,
            tc.tile_pool(name=f"l{l}_pps", bufs=2, space="PSUM") as pps,
            tc.tile_pool(name=f"l{l}_ppsb", bufs=1, space="PSUM") as ppsb,
        ):
            # load all q/k weight col-tiles once (shared by pos + qkv proj)
            qkwt = []
            for proj in range(2):
                row = []
                for jt in range(JT):
                    wof = proj * c.DCL + jt * 128
                    wt = wqkp.tile([128, DT, 128], BF16, tag=f"wqk{proj}{jt}",
                                   name=f"wqk{proj}{jt}")
                    nc.sync.dma_start(wt[:], wqkv.ap()[l, :, :, wof:wof + 128])
                    row.append(wt)
                qkwt.append(row)

            # pos projections: pos_kT_rev from relTr/Wk, pos_qT(scaled) from
            # relT/Wq.  rel chunks loaded once per (table, chunk).
            for which, (dst, reltab, proj, pofs, scl) in enumerate(
                (
                    (poskr, relTr, 1, JT, 1.0),        # Wk part, bias bk
                    (posq, relT, 0, 0, c.scale),       # Wq part, bias bq*s
                )
            ):
                rts = []
                for uc in range(c.NUC):
                    rt = relp.tile([128, DT, c.CU], BF16, tag=f"rel{uc}")
                    nc.sync.dma_start(
                        rt[:], reltab.ap()[:, :, uc * c.CU:(uc + 1) * c.CU])
                    rts.append(rt)
                for jt in range(JT):
                    pss = [pps.tile([128, c.CU], F32, tag=f"pos{uc}", name=f"pos{uc}")
                           for uc in range(c.NUC)]
                    mm_acc_multi(nc, [p[:] for p in pss], qkwt[proj][jt][:],
                                 [r[:] for r in rts], DT, True, True)
                    for uc in range(c.NUC):
                        nc.scalar.activation(
                            dst[:, jt, 128 + uc * c.CU:128 + (uc + 1) * c.CU],
                            pss[uc][:],
                            AF.Identity, bias=bq_sb[:, pofs + jt:pofs + jt + 1],
                            scale=scl)
                nc.vector.tensor_scalar_mul(
                    dst[:, :, 0:128],
                    dst[:, :, 128:129].to_broadcast((128, JT, 128)), 1.0)
                nc.vector.tensor_scalar_mul(
                    dst[:, :, PW - 128:PW],
                    dst[:, :, PW - 129:PW - 128].to_broadcast((128, JT, 128)),
                    1.0)

            # qkv projections (feature-major q/k; token-major v),
            # chunk-outer so chunk 0's projections start right after LN2's
            # chunk 0 instead of waiting for the whole LN.
            for ch in range(NCH):
                for proj in range(2):  # 0=q, 1=k
                    dst = (qsT, kT)[proj]
                    scl = (c.scale, 1.0)[proj]
                    for jt in range(JT):
                        ps = ppsb.tile([128, CH], F32, tag=f"qkv{ch}",
                                       name=f"qkv{ch}")
                        mm_acc(nc, ps[:], qkwt[proj][jt][:],
                               hTbf[:, :, ch * CH:(ch + 1) * CH], DT,
                               True, True)
                        nc.scalar.activation(
                            dst[:, jt, ch * CH:(ch + 1) * CH], ps[:],
                            AF.Identity,
                            bias=bq_sb[:, proj * JT + jt:proj * JT + jt + 1],
                            scale=scl)
            # v: out[token, dv_loc]
            wtv = wvpool.tile([128, DT, c.DCL], BF16, tag="wv")
            nc.sync.dma_start(wtv[:], wqkv.ap()[l, :, :, 2 * c.DCL:3 * c.DCL])
            for tt in range(TT):
                ps = ppsb.tile([128, c.DCL], F32, tag="vproj")
                mm_acc(nc, ps[:], hTbf[:, :, tt * 128:(tt + 1) * 128],
                       wtv[:], DT, True, True)
                for hl in range(c.NHL):
                    nc.vector.tensor_tensor(
                        v_sb[:, tt, hl * 65:hl * 65 + 64],
                        ps[:, hl * 64:(hl + 1) * 64],
                        bv_sb[:, hl * 64:(hl + 1) * 64], OP.add)
            for hl in range(c.NHL):
                nc.vector.memset(v_sb[:, :, hl * 65 + 64:hl * 65 + 65], 1.0)

        # ---- phase B: per-head attention ----
        with (
            tc.tile_pool(name=f"l{l}_ct", bufs=4) as ctp,
            tc.tile_pool(name=f"l{l}_g1", bufs=3) as g1p,
            tc.tile_pool(name=f"l{l}_g2", bufs=3) as g2p,
            tc.tile_pool(name=f"l{l}_ex", bufs=3) as exp_,
            tc.tile_pool(name=f"l{l}_sc", bufs=2) as scp,
            tc.tile_pool(name=f"l{l}_bps", bufs=1, space="PSUM") as bps,
            tc.tile_pool(name=f"l{l}_bsc", bufs=2, space="PSUM") as bsc,
            tc.tile_pool(name=f"l{l}_bp2", bufs=1, space="PSUM") as bps2,
            tc.tile_pool(name=f"l{l}_bp3", bufs=1, space="PSUM") as bps3,
        ):
            MW = 2 * c.SPAN + 256     # widened mid (covers +-128 clamp)
            ML = S - c.SPAN - 128      # mid left col in the table

            def build_tables(hl):
                """Emit table-build matmuls + staging + DRAM writes + clamp
                pads for head hl.  Returns the two DRAM table tiles."""
                jt, rb = hl // 2, 64 * (hl % 2)
                qh = qsT[rb:rb + 64, jt]      # [64, S]
                kh = kT[rb:rb + 64, jt]
                pkh = poskr[rb:rb + 64, jt]   # [64, 2*SPAN]
                pqh = posq[rb:rb + 64, jt]

                cq_dr = dramp.tile([S, c.W], FP8, tag="cq", name=f"cq{l}_{hl}")
                ck_dr = dramp.tile([S, c.W], FP8, tag="ck", name=f"ck{l}_{hl}")
                # c2p table: rows q, mid cols = q_s . pos_k_rev; p2c: rows k
                for which, (dr, lh, rh) in enumerate(
                        ((cq_dr, qh, pkh), (ck_dr, kh, pqh))):
                    th = dr[:].tensor
                    base = dr[:].offset
                    for rt in range(TT):
                        st = ctp.tile([128, MW], FP8, tag="cstage")
                        # one 1024-wide psum over the real (unclamped) rel
                        # range; the 128-col clamp flanks are broadcast on
                        # the DVE afterwards.
                        ps = bps.tile([128, 1024], F32, tag=f"ctab{which}")
                        for co in range(2):
                            nc.tensor.matmul(
                                ps[:, co * 512:(co + 1) * 512],
                                lhsT=lh[:, rt * 128:(rt + 1) * 128],
                                rhs=rh[:, 128 + co * 512:128 + (co + 1) * 512],
                                start=True, stop=True)
                        if which == 0:
                            nc.scalar.activation(
                                st[:, 128:1152], ps[:], AF.Copy, scale=256.0)
                        else:
                            nc.vector.tensor_scalar_mul(
                                st[:, 128:1152], ps[:], 256.0)
                        nc.vector.tensor_scalar_mul(
                            st[:, 0:128],
                            st[:, 128:129].to_broadcast((128, 128)), 1.0)
                        nc.vector.tensor_scalar_mul(
                            st[:, 1152:1280],
                            st[:, 1151:1152].to_broadcast((128, 128)), 1.0)
                        dst = bass.AP(
                            th, base + (rt * 128) * c.W + ML,
                            [[c.W, 128], [1, MW]])
                        nc.sync.dma_start(dst, st[:])
                    # log-doubling clamp pads (row-constant regions)
                    pos, havew = ML, 128
                    while pos > 0:
                        w = min(pos, havew)
                        ldst = bass.AP(th, base + pos - w, [[c.W, S], [1, w]])
                        lsrc = bass.AP(th, base + pos, [[c.W, S], [1, w]])
                        nc.sync.dma_start(ldst, lsrc)
                        pos -= w
                        havew += w
                    pos, havew = ML + MW, 128
                    while pos < c.W:
                        w = min(c.W - pos, havew)
                        rdst = bass.AP(th, base + pos, [[c.W, S], [1, w]])
                        rsrc = bass.AP(th, base + pos - havew,
                                       [[c.W, S], [1, w]])
                        nc.sync.dma_start(rdst, rsrc)
                        pos += w
                        havew += w
                return cq_dr, ck_dr

            def attend(hl, cq_dr, ck_dr):
                """Scores + softmax + probs@v for head hl from its tables."""
                jt, rb = hl // 2, 64 * (hl % 2)
                qh = qsT[rb:rb + 64, jt]      # [64, S]
                kh = kT[rb:rb + 64, jt]

                g1 = g1p.tile([128, TT, S], FP8, tag="g1")
                thq = cq_dr[:].tensor
                bq_ = cq_dr[:].offset
                for qt in range(TT):
                    src = bass.AP(thq, bq_ + (c.W - 1) * (qt * 128) + S - 1,
                                  [[c.W - 1, 128], [1, S]])
                    nc.sync.dma_start(g1[:, qt], src)

                ex = exp_.tile([128, TT, S], BF16, tag="ex")
                thk = ck_dr[:].tensor
                bk_ = ck_dr[:].offset
                for kt in range(TT):
                    g2 = g2p.tile([128, S], FP8, tag="g2", name=f"g2_{kt}")
                    src = bass.AP(thk, bk_ + (c.W - 1) * (kt * 128) + S,
                                  [[c.W - 1, 128], [1, S]])
                    nc.sync.dma_start(g2[:], src)
                    for ch in range(NCH):
                        ps = bsc.tile([128, CH], F32, tag="scores")
                        nc.tensor.matmul(
                            ps[:], lhsT=kh[:, kt * 128:(kt + 1) * 128],
                            rhs=qh[:, ch * CH:(ch + 1) * CH],
                            start=True, stop=False)
                        nc.tensor.matmul(
                            ps[:], lhsT=ident8[:],
                            rhs=g2[:, ch * CH:(ch + 1) * CH],
                            start=False, stop=False)
                        nq = CH // 128
                        for qi in range(nq):
                            qt = ch * nq + qi
                            nc.tensor.matmul(
                                ps[:, qi * 128:(qi + 1) * 128],
                                lhsT=g1[:, qt, kt * 128:(kt + 1) * 128],
                                rhs=ident8[:],
                                start=False, stop=True,
                                skip_group_check=(qi != nq - 1))
                        nc.scalar.activation(
                            ex[:, kt, ch * CH:(ch + 1) * CH], ps[:], AF.Exp,
                            bias=mb_sb[:, kt:kt + 1], scale=1.0)

                for ch in range(NCH):
                    pv = bps2.tile([65, CH], F32, tag="pv")
                    for kt in range(TT):
                        nc.tensor.matmul(
                            pv[:], lhsT=v_sb[:, kt, hl * 65:hl * 65 + 65],
                            rhs=ex[:, kt, ch * CH:(ch + 1) * CH],
                            start=(kt == 0), stop=(kt == TT - 1))
                    rec = scp.tile([1, CH], BF16, tag="rec")
                    with nc.allow_low_precision(reason="softmax denom bf16"):
                        nc.vector.reciprocal(rec[:], pv[64:65, :])
                    pb = bps3.tile([64, CH], F32, tag="recb")
                    nc.tensor.matmul(pb[:], lhsT=ones1x64[:], rhs=rec[:],
                                     start=True, stop=True)
                    rb_sb = scp.tile([64, CH], F32, tag="recbs")
                    nc.scalar.copy(rb_sb[:], pb[:])
                    nc.vector.tensor_tensor(
                        ctxT[rb:rb + 64, jt, ch * CH:(ch + 1) * CH],
                        pv[0:64, :], rb_sb[:], OP.mult)

            # software-pipeline heads: the PE queue is in-order, so head
            # h's score matmuls must not sit at the queue head while h's
            # table DRAM roundtrip is still in flight — keep LOOKAHEAD
            # heads of table builds queued ahead.
            LOOKAHEAD = 2
            built = {}
            for j in range(min(LOOKAHEAD + 1, c.NHL)):
                built[j] = build_tables(j)
            for hl in range(c.NHL):
                attend(hl, *built.pop(hl))
                nxt = hl + LOOKAHEAD + 1
                if nxt < c.NHL:
                    built[nxt] = build_tables(nxt)

        attn_scope.__exit__(None, None, None)   # free qsT/kT/v/pos SBUF

        # ---- phase C: Wo + AR + LN1 ----
        # per-token-chunk AR tensors: chunk 0's reduce+LN runs while chunk 1
        # is still accumulating
        ar1 = [dramp.tile([128, DT, CH], BF16, tag=f"arin{ch}",
                          name=f"ar1i_{l}_{ch}") for ch in range(NCH)]
        ar1o = [dramp.tile([128, DT, CH], BF16, tag=f"arout{ch}",
                           name=f"ar1o_{l}_{ch}") for ch in range(NCH)]
        with (
            tc.tile_pool(name=f"l{l}_wops", bufs=1, space="PSUM") as wops,
            tc.tile_pool(name=f"l{l}_wost", bufs=3) as wost,
            tc.tile_pool(name=f"l{l}_wo", bufs=1) as wopool,
        ):
            wos = wopool.tile([128, JT, D], BF16, tag="wo")
            nc.sync.dma_start(wos[:], wo.ap()[l])
            # chunk-outer so chunk 0's AllReduce fires while chunk 1 runs
            for ch in range(NCH):
                for dt in range(DT):
                    ps = wops.tile([128, CH], F32, tag=f"wo{ch}",
                                   name=f"wo{ch}")
                    mm_acc(nc, ps[:], wos[:, :, dt * 128:(dt + 1) * 128],
                           ctxT[:, :, ch * CH:(ch + 1) * CH], JT, True, False)
                    nc.tensor.matmul(
                        ps[:], lhsT=bo_sb[:, dt * 128:(dt + 1) * 128],
                        rhs=onesrow[:], start=False, stop=True)
                    st = wost.tile([128, CH], BF16, tag="wost")
                    nc.vector.scalar_tensor_tensor(
                        st[:], hTbf[:, dt, ch * CH:(ch + 1) * CH], 0.5,
                        ps[:], OP.mult, OP.add)
                    nc.sync.dma_start(ar1[ch][:, dt], st[:])
                if c.n_cores == 1 or c.no_cc:
                    nc.sync.dma_start(ar1o[ch][:], ar1[ch][:])
                else:
                    nc.gpsimd.collective_compute(
                        "AllReduce", OP.add, replica_groups=pairs,
                        ins=[ar1[ch].opt()], outs=[ar1o[ch].opt()])
        _ln(nc, tc, c, l, ar1o, hTbf, ln1g, ln1b, onesb, eps2,
            invD_row, D_row, onesf)

        # ---- phase D: FFN + AR + LN2 ----
        ar2 = [dramp.tile([128, DT, CH], BF16, tag=f"arin{ch}",
                          name=f"ar2i_{l}_{ch}") for ch in range(NCH)]
        ar2o = [dramp.tile([128, DT, CH], BF16, tag=f"arout{ch}",
                           name=f"ar2o_{l}_{ch}") for ch in range(NCH)]
        with (
            tc.tile_pool(name=f"l{l}_gt", bufs=2) as gtp,
            tc.tile_pool(name=f"l{l}_w1", bufs=1) as w1pool,
            tc.tile_pool(name=f"l{l}_w2", bufs=1) as w2pool,
            tc.tile_pool(name=f"l{l}_f1ps", bufs=1, space="PSUM") as f1ps,
            tc.tile_pool(name=f"l{l}_f2ps", bufs=1, space="PSUM") as f2ps,
            tc.tile_pool(name=f"l{l}_fst", bufs=3) as fst,
        ):
            w1t = w1pool.tile([128, DT, c.FL], BF16, tag="w1")
            nc.sync.dma_start(w1t[:], w1.ap()[l])
            w2t = w2pool.tile([128, FT, D], BF16, tag="w2")
            nc.sync.dma_start(w2t[:], w2.ap()[l])
            gts = [gtp.tile([128, FT, CH], BF16, tag="gt", name=f"gt{ch}")
                   for ch in range(NCH)]
            # chunk-outer: chunk 0 flows W1 -> W2 -> AR while chunk 1
            # computes, so the AllReduce latency overlaps compute.
            for ch in range(NCH):
                hchunk = hTbf[:, :, ch * CH:(ch + 1) * CH]
                for ft in range(FT):
                    ps = f1ps.tile([128, CH], F32, tag=f"f1{ch}",
                                   name=f"f1{ch}")
                    mm_acc(nc, ps[:], w1t[:, :, ft * 128:(ft + 1) * 128],
                           hchunk, DT, True, True)
                    nc.scalar.activation(
                        gts[ch][:, ft], ps[:],
                        AF.Gelu if c.act == "gelu" else AF.Relu,
                        bias=b1_sb[:, ft:ft + 1], scale=1.0)
                for dt in range(DT):
                    ps = f2ps.tile([128, CH], F32, tag=f"f2{ch}",
                                   name=f"f2{ch}")
                    mm_acc(nc, ps[:], w2t[:, :, dt * 128:(dt + 1) * 128],
                           gts[ch][:], FT, True, False)
                    nc.tensor.matmul(
                        ps[:], lhsT=b2_sb[:, dt * 128:(dt + 1) * 128],
                        rhs=onesrow[:], start=False, stop=True)
                    st = fst.tile([128, CH], BF16, tag="fst")
                    nc.vector.scalar_tensor_tensor(
                        st[:], hTbf[:, dt, ch * CH:(ch + 1) * CH], 0.5,
                        ps[:], OP.mult, OP.add)
                    nc.sync.dma_start(ar2[ch][:, dt], st[:])
                if c.n_cores == 1 or c.no_cc:
                    nc.sync.dma_start(ar2o[ch][:], ar2[ch][:])
                else:
                    nc.gpsimd.collective_compute(
                        "AllReduce", OP.add, replica_groups=pairs,
                        ins=[ar2[ch].opt()], outs=[ar2o[ch].opt()])
        _ln(nc, tc, c, l, ar2o, hTbf, ln2g, ln2b, onesb, eps2,
            invD_row, D_row, onesf)


def _ln(nc, tc, c, l, x_drs, hTbf, g_in, b_in, onesb, eps2,
        invD_row, D_row, onesf):
    """Feature-major layernorm over partitions: x in per-chunk DRAM tiles
    [128, DT, CH] bf16 -> hTbf.  Single pass over x: tiles are kept in SBUF
    between the stats accumulation and the normalize step.
    rstd computed as D/sqrt(D*s1 - s0^2 + D^2*eps)."""
    S, CH, NCH, DT, SUB = c.S, c.CH, c.NCH, c.DT, c.SUB
    with (
        tc.tile_pool(name=f"ln{l}", bufs=2) as lp,
        tc.tile_pool(name=f"ln{l}s", bufs=1) as lps,
        tc.tile_pool(name=f"ln{l}ps", bufs=1, space="PSUM") as pps,
        tc.tile_pool(name=f"ln{l}pb", bufs=2, space="PSUM") as pbs,
    ):
        g_sb = lps.tile([128, DT], F32, tag="g")
        nc.sync.dma_start(g_sb[:], g_in.ap()[l])
        b_sb = lps.tile([128, DT], F32, tag="b")
        nc.sync.dma_start(b_sb[:], b_in.ap()[l])

        stats0 = pps.tile([1, S], F32, tag="stats0")
        stats1 = pps.tile([1, S], F32, tag="stats1")
        ngr = DT // SUB
        s0 = lps.tile([1, S], F32, tag="s0")
        s0b = lps.tile([1, S], BF16, tag="s0b")
        s1 = lps.tile([1, S], F32, tag="s1")
        u = lps.tile([1, S], F32, tag="u")
        rp = lps.tile([1, S], BF16, tag="rp")
        mu_b = lps.tile([128, S], F32, tag="mub")
        rs_b = lps.tile([128, S], F32, tag="rsb")
        # fully per-chunk: stats(ch) -> rstd(ch) -> bcast(ch) ->
        # normalize(ch) before chunk ch+1's stats, so chunk 0's output
        # unblocks downstream consumers while chunk 1 still reduces.
        for ch in range(NCH):
            xts = {}
            for g in range(ngr):
                xt = lps.tile([128, SUB, CH], BF16, tag=f"x{ch}_{g}")
                xts[g] = xt
                nc.sync.dma_start(
                    xt[:], x_drs[ch][:, g * SUB:(g + 1) * SUB])
                x2 = lp.tile([128, SUB, CH], BF16, tag="x2")
                nc.scalar.square(x2[:], xt[:])
                for s in range(SUB):
                    nc.tensor.matmul(
                        stats0[:, ch * CH:(ch + 1) * CH], lhsT=onesb[:, s],
                        rhs=xt[:, s], start=(g == 0 and s == 0),
                        stop=(g == ngr - 1 and s == SUB - 1))
                    nc.tensor.matmul(
                        stats1[:, ch * CH:(ch + 1) * CH], lhsT=onesb[:, s],
                        rhs=x2[:, s], start=(g == 0 and s == 0),
                        stop=(g == ngr - 1 and s == SUB - 1))
            cs = slice(ch * CH, (ch + 1) * CH)
            nc.scalar.copy(s0[:, cs], stats0[:, cs])
            nc.scalar.copy(s1[:, cs], stats1[:, cs])
            nc.vector.tensor_copy(s0b[:, cs], s0[:, cs])
            nc.vector.tensor_tensor(u[:, cs], s0[:, cs], s0[:, cs], OP.mult)
            nc.vector.scalar_tensor_tensor(
                u[:, cs], s1[:, cs], float(c.D), u[:, cs], OP.mult, OP.subtract)
            nc.scalar.activation(u[:, cs], u[:, cs], AF.Sqrt, bias=eps2[:],
                                 scale=1.0)
            with nc.allow_low_precision(reason="rstd bf16 broadcast"):
                nc.vector.reciprocal(rp[:, cs], u[:, cs])
            pm = pbs.tile([128, CH], F32, tag="bc")
            nc.tensor.matmul(pm[:], lhsT=invD_row[:], rhs=s0b[0:1, cs],
                             start=True, stop=True)
            nc.scalar.copy(mu_b[:, cs], pm[:])
            pr = pbs.tile([128, CH], F32, tag="bc")
            nc.tensor.matmul(pr[:], lhsT=D_row[:], rhs=rp[0:1, cs],
                             start=True, stop=True)
            nc.scalar.copy(rs_b[:, cs], pr[:])

            for g in range(ngr):
                xt = xts[g]
                xn = lp.tile([128, SUB, CH], F32, tag="xn")
                mub = mu_b[:, None, ch * CH:(ch + 1) * CH].to_broadcast(
                    (128, SUB, CH))
                nc.vector.tensor_tensor(xn[:], xt[:], mub, OP.subtract)
                rsb = rs_b[:, None, ch * CH:(ch + 1) * CH].to_broadcast(
                    (128, SUB, CH))
                nc.vector.tensor_tensor(xn[:], xn[:], rsb, OP.mult)
                for i in range(SUB):
                    dt = g * SUB + i
                    nc.scalar.activation(
                        hTbf[:, dt, ch * CH:(ch + 1) * CH], xn[:, i],
                        AF.Identity, bias=b_sb[:, dt:dt + 1],
                        scale=g_sb[:, dt:dt + 1])


# ---------------------------------------------------------------------------
# host side
# ---------------------------------------------------------------------------

def host_prep(c, inputs):
    """Build per-core in_maps from full inputs."""
    bf = ml_dtypes.bfloat16
    f32 = np.float32
    ii = {k: np.asarray(v) for k, v in inputs.items()}
    S, D, L = c.S, c.D, c.L

    def tokmaj(vec):  # [S] -> [128, TT]   t = tt*128 + p
        return np.ascontiguousarray(vec.reshape(c.TT, 128).T)

    def dpart(vec):  # [D] -> [128, DT]
        return np.ascontiguousarray(vec.reshape(c.DT, 128).T)

    rel = ii["rel_emb"].astype(f32)  # [2*SPAN, D]
    relT = np.ascontiguousarray(
        rel.T.reshape(c.DT, 128, 2 * c.SPAN).transpose(1, 0, 2)).astype(bf)
    relr = rel[::-1]
    relTr = np.ascontiguousarray(
        relr.T.reshape(c.DT, 128, 2 * c.SPAN).transpose(1, 0, 2)).astype(bf)

    in_maps = []
    for core in range(c.n_cores):
        b, half = core // 2, core % 2
        colr = slice(half * c.DCL, (half + 1) * c.DCL)
        fcol = slice(half * c.FL, (half + 1) * c.FL)

        ids = ii["input_ids"][b].astype(np.int64)
        w = np.zeros((16, S // 16), np.int16)
        for i in range(S):
            w[i % 16, i // 16] = ids[i]
        ids16 = np.tile(w, (8, 1))

        seg = ii["segment_ids"][b].astype(f32)
        mask = ii["attention_mask"][b].astype(f32)

        wq = ii["Wq"][:, :, colr].astype(f32)
        wk = ii["Wk"][:, :, colr].astype(f32)
        wv = ii["Wv"][:, :, colr].astype(f32)
        wqkv = np.concatenate([wq, wk, wv], axis=2)  # [L, D, 3*DCL]
        wqkv = wqkv.reshape(L, c.DT, 128, 3 * c.DCL).transpose(0, 2, 1, 3)

        bq = ii["bq"][:, colr].astype(f32) * c.scale
        bk = ii["bk"][:, colr].astype(f32)
        bv = ii["bv"][:, colr].astype(f32)
        bqkv = np.concatenate(
            [bq.reshape(L, c.JT, 128).transpose(0, 2, 1),
             bk.reshape(L, c.JT, 128).transpose(0, 2, 1),
             np.zeros((L, 128, c.JT), f32)], axis=2)
        bvrep = np.broadcast_to(bv[:, None, :], (L, 128, c.DCL))

        wo_ = ii["Wo"][:, colr, :].astype(f32)
        wo_ = wo_.reshape(L, c.JT, 128, D).transpose(0, 2, 1, 3)
        bo2 = (ii["bo"].astype(f32) / 2.0)[:, None, :]

        w1_ = ii["W1"][:, :, fcol].astype(f32)
        w1_ = w1_.reshape(L, c.DT, 128, c.FL).transpose(0, 2, 1, 3)
        b1_ = ii["b1"][:, fcol].astype(f32).reshape(L, c.FT, 128).transpose(0, 2, 1)
        w2_ = ii["W2"][:, fcol, :].astype(f32)
        w2_ = w2_.reshape(L, c.FT, 128, D).transpose(0, 2, 1, 3)
        b22 = (ii["b2"].astype(f32) / 2.0)[:, None, :]

        m = {
            "ids16": ids16,
            "tok_emb": ii["tok_emb"].astype(f32),
            "segsel": tokmaj(seg),
            "seg0rep": np.broadcast_to(
                ii["seg_emb"][0].astype(f32), (128, D)).copy(),
            "segdrep": np.broadcast_to(
                (ii["seg_emb"][1] - ii["seg_emb"][0]).astype(f32),
                (128, D)).copy(),
            "maskt": tokmaj(mask),
            "maskbias": tokmaj(NEG * (1.0 - mask)),
            "egrep": np.broadcast_to(
                ii["emb_ln_g"].astype(f32), (128, D)).copy(),
            "ebrep": np.broadcast_to(
                ii["emb_ln_b"].astype(f32), (128, D)).copy(),
            "relT": relT,
            "relTr": relTr,
            "wqkv": wqkv.astype(bf),
            "bqkv": np.ascontiguousarray(bqkv),
            "bvrep": np.ascontiguousarray(bvrep),
            "wo": wo_.astype(bf),
            "bo2": bo2.astype(bf),
            "w1": w1_.astype(bf),
            "b1": np.ascontiguousarray(b1_),
            "w2": w2_.astype(bf),
            "b22": b22.astype(bf),
            "ln1g": ii["ln1_g"].astype(f32).reshape(
                L, c.DT, 128).transpose(0, 2, 1),
            "ln1b": ii["ln1_b"].astype(f32).reshape(
                L, c.DT, 128).transpose(0, 2, 1),
            "ln2g": ii["ln2_g"].astype(f32).reshape(
                L, c.DT, 128).transpose(0, 2, 1),
            "ln2b": ii["ln2_b"].astype(f32).reshape(
                L, c.DT, 128).transpose(0, 2, 1),
        }
        m = {k: np.ascontiguousarray(v) for k, v in m.items()}
        in_maps.append(m)
    return in_maps


def assemble(c, results):
    """results[core]["out_hT"] [128, DT, S] -> [B, S, D] fp32."""
    out = np.zeros((c.B, c.S, c.D), np.float32)
    for b in range(c.B):
        hT = results[2 * b]["out_hT"]  # [128, DT, S]
        out[b] = hT.transpose(2, 1, 0).reshape(c.S, c.D)
    return out


_nc_cache = {}


def _get_nc(c):
    key = (c.B, c.S, c.D, c.H, c.F, c.L, c.V, c.SPAN, c.n_cores)
    if key not in _nc_cache:
        _nc_cache[key] = build_nc(c)
    return _nc_cache[key]


def kernel(**inputs):
    from concourse import bass_utils
    c = Cfg()
    nc = _get_nc(c)
    in_maps = host_prep(c, inputs)
    res = bass_utils.run_bass_kernel_spmd(
        nc, in_maps, core_ids=list(range(c.n_cores)))
    return assemble(c, res.results)

